# revision 14
# baseline (speedup 1.0000x reference)
"""TRN2 Bass kernel for nn_Attention_76802605187492.

Math (B=64, T=512, H=1024, A=300):
  The aspect branch (aspect, W_v, b_v, w_w[:, H:], w_b) only adds a
  per-batch constant to the attention scores, which softmax cancels, so it
  does not affect the output at all.  What remains per batch b:
    scores[t] = u . tanh(W_h hidden[b,t] + b_h)      u = w_w[0, :H]
    alpha     = softmax_t(scores)
    r         = sum_t alpha[t] hidden[b,t]
    p_b       = r @ W_p.T
    x_j       = hidden[j,-1] @ W_x.T                  (all j)
    out[b,j]  = tanh(p_b + x_j + (b_p + b_x))         -> [B, B, H]

Sharding: data-parallel over batch across 8 cores (8 batches each). Each
core computes p for its batches, x for all 64 (tiny), and emits the
[8, 64, 1024] output slab.

All PE matmuls are bf16. The only output-critical matmul is the x term
(it dominates the pre-tanh activation), so it is computed in split
precision: x = hi@hi + lo@hi + hi@lo with hi/lo bf16 halves of the fp32
operands, accumulated in fp32 PSUM (error ~1e-5). b_p + b_x rides the
same accumulation via k=1 ones-matmuls, also in hi+lo halves.

Engine-AP partition bases must be 0/32/64(/96), so:
  - scores live on partition 0 as [1, 4096]; a SBUF->SBUF DMA reshapes
    them to [8, 512] (DMA has no partition-base restriction);
  - r for all 8 batches accumulates into ONE [8, 512] psum pair using
    per-batch column-masked alphaT tiles (garbage rows vanish because the
    masked columns are zero), so no per-row psum extraction is needed.

Final stage per output tile [128=(2 local-i x 64 j), 512]:
  psum = A_sel @ p   (A_sel constant 0/1 selector, k=8)
  out  = tanh(psum + x2)   with x2 = x duplicated on both partition halves
"""

import sys

sys.path.insert(0, "/opt/trn_rl_repo")
sys.path.insert(0, "/opt/trn_rl_repo/concourse")

import numpy as np
import ml_dtypes

import concourse.bass as bass
import concourse.mybir as mybir
from concourse import tile
from concourse.bass_utils import run_bass_kernel_spmd

F32 = mybir.dt.float32
BF16 = mybir.dt.bfloat16
BF16_NP = ml_dtypes.bfloat16
TANH = mybir.ActivationFunctionType.Tanh
EXP = mybir.ActivationFunctionType.Exp

B, T, H = 64, 512, 1024
NCORES = 8
PB = B // NCORES          # batches per core = 8
R = PB * T                # rows per core = 4096
KT = H // 128             # 8 k-tiles over h_in
MT = H // 128             # 8 m-tiles over h_out
TT = T // 128             # 4 t-tiles per batch

_CACHE: dict = {}


def _build_nc() -> bass.Bass:
    nc = bass.Bass()

    xT = nc.declare_dram_parameter("xT", [H, R], BF16, isOutput=False)
    hnat = nc.declare_dram_parameter("hnat", [R, H], BF16, isOutput=False)
    whT = nc.declare_dram_parameter("whT", [H, H], BF16, isOutput=False)
    bh = nc.declare_dram_parameter("bh", [128, MT], F32, isOutput=False)
    uu = nc.declare_dram_parameter("u", [128, MT], BF16, isOutput=False)
    wpT = nc.declare_dram_parameter("wpT", [H, H], BF16, isOutput=False)
    wxh = nc.declare_dram_parameter("wxT_hi", [H, H], BF16, isOutput=False)
    wxl = nc.declare_dram_parameter("wxT_lo", [H, H], BF16, isOutput=False)
    hlh = nc.declare_dram_parameter("hlastT_hi", [H, B], BF16, isOutput=False)
    hll = nc.declare_dram_parameter("hlastT_lo", [H, B], BF16, isOutput=False)
    selA = nc.declare_dram_parameter("selA", [PB, 4, 128], BF16, isOutput=False)
    bpx = nc.declare_dram_parameter("bpx", [1, 2 * H], BF16, isOutput=False)
    ones = nc.declare_dram_parameter("ones", [1, B], BF16, isOutput=False)
    ident = nc.declare_dram_parameter("ident", [PB, PB], BF16, isOutput=False)
    out = nc.declare_dram_parameter("out", [PB, B, H], F32, isOutput=True)

    with tile.TileContext(nc) as tc:
        with (
            tc.tile_pool(name="const", bufs=1) as cp,
            tc.tile_pool(name="xchunk", bufs=2) as xp,
            tc.tile_pool(name="tz", bufs=10) as tzp,
            tc.tile_pool(name="hb", bufs=4) as hbp,
            tc.tile_pool(name="small", bufs=1) as sp,
            tc.tile_pool(name="outp", bufs=2) as op_,
            tc.tile_pool(name="ps", bufs=6, space=bass.MemorySpace.PSUM) as pp,
            tc.tile_pool(name="tps", bufs=2, space=bass.MemorySpace.PSUM) as tpp,
        ):
            # ---- resident constants for phase A (sync DMA queue) ----
            w_sb = cp.tile([128, KT, H], BF16)
            nc.sync.dma_start(
                w_sb[:], whT[:].rearrange("(kt p) n -> p kt n", p=128)
            )
            bh_sb = cp.tile([128, MT], F32)
            nc.sync.dma_start(bh_sb[:], bh[:])
            u_sb = cp.tile([128, MT], BF16)
            nc.sync.dma_start(u_sb[:], uu[:])
            id_sb = cp.tile([PB, PB], BF16)
            nc.sync.dma_start(id_sb[:], ident[:])

            # scores for all 8 local batches, on partition 0
            scores_sb = sp.tile([1, R], F32)

            # ---- phase A: big matmul + scores, one chunk per batch ----
            for b in range(PB):
                xc = xp.tile([128, KT, T], BF16)
                nc.gpsimd.dma_start(
                    xc[:],
                    xT[:, b * T : (b + 1) * T].rearrange("(kt p) n -> p kt n", p=128),
                )
                tz_tiles = []
                for m in range(MT):
                    z_ps = pp.tile([128, T], F32, tag="ps")
                    for kt in range(KT):
                        nc.tensor.matmul(
                            z_ps[:],
                            w_sb[:, kt, m * 128 : (m + 1) * 128],
                            xc[:, kt, :],
                            start=(kt == 0),
                            stop=(kt == KT - 1),
                        )
                    tz = tzp.tile([128, T], BF16)
                    nc.scalar.activation(tz[:], z_ps[:], TANH, bias=bh_sb[:, m : m + 1])
                    tz_tiles.append(tz)
                s_ps = pp.tile([1, T], F32, tag="ps")
                for m in range(MT):
                    nc.tensor.matmul(
                        s_ps[:1, :],
                        u_sb[:, m : m + 1],
                        tz_tiles[m][:],
                        start=(m == 0),
                        stop=(m == MT - 1),
                    )
                nc.scalar.copy(scores_sb[:1, b * T : (b + 1) * T], s_ps[:1, :])

            # ---- late-loaded constants (sync queue, drain during phase A) ----
            wpT_sb = cp.tile([128, KT, H], BF16)
            nc.sync.dma_start(wpT_sb[:], wpT[:].rearrange("(kt p) n -> p kt n", p=128))
            wxh_sb = cp.tile([128, KT, H], BF16)
            nc.sync.dma_start(wxh_sb[:], wxh[:].rearrange("(kt p) n -> p kt n", p=128))
            wxl_sb = cp.tile([128, KT, H], BF16)
            nc.sync.dma_start(wxl_sb[:], wxl[:].rearrange("(kt p) n -> p kt n", p=128))
            hlh_sb = cp.tile([128, KT, B], BF16)
            nc.sync.dma_start(hlh_sb[:], hlh[:].rearrange("(kt p) j -> p kt j", p=128))
            hll_sb = cp.tile([128, KT, B], BF16)
            nc.sync.dma_start(hll_sb[:], hll[:].rearrange("(kt p) j -> p kt j", p=128))
            selA_sb = cp.tile([PB, 4, 128], BF16)
            nc.sync.dma_start(selA_sb[:], selA[:])
            bpx_sb = cp.tile([1, 2 * H], BF16)
            nc.sync.dma_start(bpx_sb[:], bpx[:])
            ones_sb = cp.tile([1, B], BF16)
            nc.sync.dma_start(ones_sb[:], ones[:])

            # ---- phase F: x2 = (hlast @ W_x.T + b_p + b_x) in split bf16 ----
            x2_sb = sp.tile([128, H], F32)
            for hc in range(2):
                x_ps = pp.tile([B, 512], F32, tag="ps")
                n = 0
                terms = [(hlh_sb, wxh_sb), (hll_sb, wxh_sb), (hlh_sb, wxl_sb)]
                nmm = len(terms) * KT + 2
                for lh, rh in terms:
                    for kt in range(KT):
                        nc.tensor.matmul(
                            x_ps[:],
                            lh[:, kt, :],
                            rh[:, kt, hc * 512 : (hc + 1) * 512],
                            start=(n == 0),
                            stop=(n == nmm - 1),
                        )
                        n += 1
                for row in range(2):
                    nc.tensor.matmul(
                        x_ps[:],
                        ones_sb[:1, :],
                        bpx_sb[:1, row * H + hc * 512 : row * H + (hc + 1) * 512],
                        start=(n == 0),
                        stop=(n == nmm - 1),
                    )
                    n += 1
                nc.scalar.copy(x2_sb[:B, hc * 512 : (hc + 1) * 512], x_ps[:])
                nc.scalar.copy(x2_sb[B:, hc * 512 : (hc + 1) * 512], x_ps[:])

            # ---- reshape scores to [8, 512] via SBUF->SBUF DMA (gpsimd
            #      queue so it cannot head-of-line block the hb loads) ----
            scores8 = sp.tile([PB, T], F32)
            nc.gpsimd.dma_start(scores8[:], scores_sb[:1, :])

            # ---- softmax over t ----
            e8 = sp.tile([PB, T], F32)
            nc.scalar.activation(e8[:], scores8[:], EXP)
            esum = sp.tile([PB, 1], F32)
            nc.vector.reduce_sum(esum[:], e8[:], axis=mybir.AxisListType.X)
            einv = sp.tile([PB, 1], F32)
            nc.vector.reciprocal(einv[:], esum[:])
            alpha8 = sp.tile([PB, T], BF16)
            nc.vector.tensor_scalar_mul(alpha8[:], e8[:], einv[:, :1])

            # ---- transpose alpha and build per-batch column-masked tiles ----
            am_sb = sp.tile([128, TT, PB, PB], BF16)
            nc.vector.memset(am_sb[:], 0.0)
            for kt in range(TT):
                t_ps = tpp.tile([128, PB], BF16, tag="tp")
                nc.tensor.transpose(
                    t_ps[:], alpha8[:, kt * 128 : (kt + 1) * 128], id_sb[:]
                )
                for bb in range(PB):
                    nc.scalar.copy(am_sb[:, kt, bb, bb : bb + 1], t_ps[:, bb : bb + 1])

            # ---- phase C: r for all batches into one psum pair ----
            r_ps = [pp.tile([PB, 512], F32, tag="ps", name=f"r_ps{i}") for i in range(2)]
            nlast = PB * TT
            n = 0
            for bb in range(PB):
                hb_t = hbp.tile([128, TT, H], BF16)
                nc.sync.dma_start(
                    hb_t[:],
                    hnat[bb * T : (bb + 1) * T, :].rearrange(
                        "(kt p) h -> p kt h", p=128
                    ),
                )
                for kt in range(TT):
                    for hc in range(2):
                        nc.tensor.matmul(
                            r_ps[hc][:],
                            am_sb[:, kt, bb, :],
                            hb_t[:, kt, hc * 512 : (hc + 1) * 512],
                            start=(n == 0),
                            stop=(n == nlast - 1),
                        )
                    n += 1
            rflat_bf = sp.tile([PB, H], BF16)
            for hc in range(2):
                nc.scalar.copy(rflat_bf[:, hc * 512 : (hc + 1) * 512], r_ps[hc][:])

            # ---- phase D: transpose r -> rT [h_in, i] ----
            rT_sb = sp.tile([128, KT, PB], BF16)
            for mt in range(KT):
                t_ps = tpp.tile([128, PB], BF16, tag="tp")
                nc.tensor.transpose(
                    t_ps[:], rflat_bf[:, mt * 128 : (mt + 1) * 128], id_sb[:]
                )
                nc.scalar.copy(rT_sb[:, mt, :], t_ps[:])

            # ---- phase E: p = r @ W_p.T ----
            p_sb = sp.tile([PB, H], BF16)
            for hc in range(2):
                p_ps = pp.tile([PB, 512], F32, tag="ps")
                for kt in range(KT):
                    nc.tensor.matmul(
                        p_ps[:],
                        rT_sb[:, kt, :],
                        wpT_sb[:, kt, hc * 512 : (hc + 1) * 512],
                        start=(kt == 0),
                        stop=(kt == KT - 1),
                    )
                nc.scalar.copy(p_sb[:, hc * 512 : (hc + 1) * 512], p_ps[:])

            # ---- phase G: out = tanh(A_sel @ p + x2) ----
            for q in range(4):
                for hc in range(2):
                    o_ps = pp.tile([128, 512], F32, tag="ps")
                    nc.tensor.matmul(
                        o_ps[:],
                        selA_sb[:, q, :],
                        p_sb[:, hc * 512 : (hc + 1) * 512],
                        start=True,
                        stop=True,
                    )
                    o_sb = op_.tile([128, 512], F32, tag="oadd")
                    nc.vector.tensor_add(
                        o_sb[:], o_ps[:], x2_sb[:, hc * 512 : (hc + 1) * 512]
                    )
                    o_sb2 = op_.tile([128, 512], F32, tag="otanh")
                    nc.scalar.activation(o_sb2[:], o_sb[:], TANH)
                    nc.sync.dma_start(
                        out[2 * q : 2 * q + 2, :, hc * 512 : (hc + 1) * 512].rearrange(
                            "i j h -> (i j) h"
                        ),
                        o_sb2[:],
                    )
    _split_excess_waits(nc)
    return nc


def _split_excess_waits(nc: bass.Bass, max_waits: int = 1) -> None:
    """Walrus's per-instruction sync-wait slots are limited; move excess
    on_wait entries onto wait-only NoOps inserted just before the
    instruction (same engine, so ordering is preserved)."""
    for fn in nc.m.functions:
        for blk in fn.blocks:
            new = []
            for inst in blk.instructions:
                si = inst.sync_info
                waits = list(si.on_wait) if si is not None and si.on_wait else []
                if len(waits) > max_waits:
                    extra, keep = waits[:-max_waits], waits[-max_waits:]
                    for ci in range(0, len(extra), max_waits):
                        nop = mybir.InstNoOp(
                            name=f"{inst.name}-wsplit{ci}", ins=[], outs=[]
                        )
                        nop.engine = inst.engine
                        nop.sync_info = mybir.SyncInfo(
                            on_wait=extra[ci : ci + max_waits], on_update=[]
                        )
                        new.append(nop)
                    inst.sync_info = mybir.SyncInfo(
                        on_wait=keep, on_update=list(si.on_update or [])
                    )
                new.append(inst)
            blk.instructions[:] = new


def _split_bf16(a: np.ndarray) -> tuple[np.ndarray, np.ndarray]:
    hi = a.astype(BF16_NP)
    lo = (a - hi.astype(np.float32)).astype(BF16_NP)
    return hi, lo


def _host_prep(inputs: dict) -> list[dict]:
    hidden = np.asarray(inputs["hidden"], np.float32)
    W_h = np.asarray(inputs["W_h"], np.float32)
    b_h = np.asarray(inputs["b_h"], np.float32)
    w_w = np.asarray(inputs["w_w"], np.float32)
    W_p = np.asarray(inputs["W_p"], np.float32)
    b_p = np.asarray(inputs["b_p"], np.float32)
    W_x = np.asarray(inputs["W_x"], np.float32)
    b_x = np.asarray(inputs["b_x"], np.float32)

    selA = np.zeros((PB, 4, 128), np.float32)
    for q in range(4):
        for m in range(128):
            selA[2 * q + m // 64, q, m] = 1.0

    wxT = np.ascontiguousarray(W_x.T)
    wx_hi, wx_lo = _split_bf16(wxT)
    hlT = np.ascontiguousarray(hidden[:, -1, :].T)
    hl_hi, hl_lo = _split_bf16(hlT)
    bpx_hi, bpx_lo = _split_bf16((b_p + b_x).reshape(1, H))

    shared = {
        "whT": np.ascontiguousarray(W_h.T).astype(BF16_NP),
        "bh": np.ascontiguousarray(b_h.reshape(MT, 128).T),
        "u": np.ascontiguousarray(w_w[0, :H].reshape(MT, 128).T).astype(BF16_NP),
        "wpT": np.ascontiguousarray(W_p.T).astype(BF16_NP),
        "wxT_hi": wx_hi,
        "wxT_lo": wx_lo,
        "hlastT_hi": hl_hi,
        "hlastT_lo": hl_lo,
        "selA": selA.astype(BF16_NP),
        "bpx": np.concatenate([bpx_hi, bpx_lo], axis=1),
        "ones": np.ones((1, B), BF16_NP),
        "ident": np.eye(PB, dtype=np.float32).astype(BF16_NP),
    }

    in_maps = []
    for c in range(NCORES):
        flat = hidden[c * PB : (c + 1) * PB].reshape(R, H)
        m = dict(shared)
        m["xT"] = np.ascontiguousarray(flat.T).astype(BF16_NP)
        m["hnat"] = flat.astype(BF16_NP)
        in_maps.append(m)
    return in_maps


def _ensure_ntff_hook() -> None:
    """The agent image's antenv lacks axon_hooks; register a shim module
    wired to the libaxon NTFF profile hook so trace=True works."""
    try:
        from antenv.axon_hooks import get_axon_ntff_profile_hook  # noqa: F401
        return
    except ImportError:
        pass
    import types
    import antenv
    from trn_agent_boot.trn_boot import _ntff_profile_via_ctypes

    mod = types.ModuleType("antenv.axon_hooks")
    holder = {"hook": _ntff_profile_via_ctypes("/opt/axon/libaxon_pjrt.so")}
    mod.get_axon_ntff_profile_hook = lambda: holder["hook"]
    mod.set_axon_ntff_profile_hook = lambda h: holder.__setitem__("hook", h)
    sys.modules["antenv.axon_hooks"] = mod
    antenv.axon_hooks = mod


def run(inputs: dict, trace: bool = False, **kw):
    if trace:
        _ensure_ntff_hook()
    if "nc" not in _CACHE:
        _CACHE["nc"] = _build_nc()
    nc = _CACHE["nc"]
    in_maps = _host_prep(inputs)
    res = run_bass_kernel_spmd(nc, in_maps, list(range(NCORES)), trace=trace, **kw)
    out = np.empty((B, B, H), np.float32)
    for c in range(NCORES):
        out[c * PB : (c + 1) * PB] = np.asarray(res.results[c]["out"], np.float32)
    return out, res


def kernel(**inputs) -> np.ndarray:
    out, _ = run(inputs)
    return out


# revision 19
# speedup vs baseline: 1.0524x; 1.0524x over previous
"""TRN2 Bass kernel for nn_Attention_76802605187492.

Math (B=64, T=512, H=1024, A=300):
  The aspect branch (aspect, W_v, b_v, w_w[:, H:], w_b) only adds a
  per-batch constant to the attention scores, which softmax cancels, so it
  does not affect the output at all.  What remains per batch b:
    scores[t] = u . tanh(W_h hidden[b,t] + b_h)      u = w_w[0, :H]
    alpha     = softmax_t(scores)
    r         = sum_t alpha[t] hidden[b,t]
    p_b       = r @ W_p.T
    x_j       = hidden[j,-1] @ W_x.T                  (all j)
    out[b,j]  = tanh(p_b + x_j + (b_p + b_x))         -> [B, B, H]

Sharding: data-parallel over batch across 8 cores (8 batches each). Each
core computes p for its batches, x for all 64 (tiny), and emits the
[8, 64, 1024] output slab.

All PE matmuls are bf16. The only output-critical matmul is the x term
(it dominates the pre-tanh activation), so it is computed in split
precision: x = hi@hi + lo@hi + hi@lo with hi/lo bf16 halves of the fp32
operands, accumulated in fp32 PSUM (error ~1e-5). b_p + b_x rides the
same accumulation via k=1 ones-matmuls, also in hi+lo halves.

Engine-AP partition bases must be 0/32/64(/96), so:
  - scores live on partition 0 as [1, 4096]; a SBUF->SBUF DMA reshapes
    them to [8, 512] (DMA has no partition-base restriction);
  - r for all 8 batches accumulates into ONE [8, 512] psum pair using
    per-batch column-masked alphaT tiles (garbage rows vanish because the
    masked columns are zero), so no per-row psum extraction is needed.

Final stage per output tile [128=(2 local-i x 64 j), 512]:
  psum = A_sel @ p   (A_sel constant 0/1 selector, k=8)
  out  = tanh(psum + x2)   with x2 = x duplicated on both partition halves
"""

import sys

sys.path.insert(0, "/opt/trn_rl_repo")
sys.path.insert(0, "/opt/trn_rl_repo/concourse")

import numpy as np
import ml_dtypes

import concourse.bass as bass
import concourse.mybir as mybir
from concourse import tile
from concourse.bass_utils import run_bass_kernel_spmd

F32 = mybir.dt.float32
BF16 = mybir.dt.bfloat16
BF16_NP = ml_dtypes.bfloat16
TANH = mybir.ActivationFunctionType.Tanh
EXP = mybir.ActivationFunctionType.Exp

B, T, H = 64, 512, 1024
NCORES = 8
PB = B // NCORES          # batches per core = 8
R = PB * T                # rows per core = 4096
KT = H // 128             # 8 k-tiles over h_in
MT = H // 128             # 8 m-tiles over h_out
TT = T // 128             # 4 t-tiles per batch

_CACHE: dict = {}


def _build_nc() -> bass.Bass:
    nc = bass.Bass()

    xT = nc.declare_dram_parameter("xT", [H, R], BF16, isOutput=False)
    hnat = nc.declare_dram_parameter("hnat", [R, H], BF16, isOutput=False)
    whT8 = nc.declare_dram_parameter("whT8", [MT, 128, KT * 128], BF16, isOutput=False)
    bh = nc.declare_dram_parameter("bh", [128, MT], F32, isOutput=False)
    uu = nc.declare_dram_parameter("u", [128, MT], BF16, isOutput=False)
    wpT = nc.declare_dram_parameter("wpT", [H, H], BF16, isOutput=False)
    wxh = nc.declare_dram_parameter("wxT_hi", [H, H], BF16, isOutput=False)
    wxl = nc.declare_dram_parameter("wxT_lo", [H, H], BF16, isOutput=False)
    hlh = nc.declare_dram_parameter("hlastT_hi", [H, B], BF16, isOutput=False)
    hll = nc.declare_dram_parameter("hlastT_lo", [H, B], BF16, isOutput=False)
    selA = nc.declare_dram_parameter("selA", [PB, 4, 128], BF16, isOutput=False)
    bpx = nc.declare_dram_parameter("bpx", [1, 2 * H], BF16, isOutput=False)
    ones = nc.declare_dram_parameter("ones", [1, B], BF16, isOutput=False)
    ident = nc.declare_dram_parameter("ident", [PB, PB], BF16, isOutput=False)
    out = nc.declare_dram_parameter("out", [PB, B, H], F32, isOutput=True)

    with tile.TileContext(nc) as tc:
        with (
            tc.tile_pool(name="const", bufs=1) as cp,
            tc.tile_pool(name="xchunk", bufs=2) as xp,
            tc.tile_pool(name="tz", bufs=10) as tzp,
            tc.tile_pool(name="hb", bufs=4) as hbp,
            tc.tile_pool(name="small", bufs=1) as sp,
            tc.tile_pool(name="sc", bufs=2) as scp,
            tc.tile_pool(name="outp", bufs=4) as op_,
            tc.tile_pool(name="ps", bufs=6, space=bass.MemorySpace.PSUM) as pp,
            tc.tile_pool(name="tps", bufs=2, space=bass.MemorySpace.PSUM) as tpp,
        ):
            # ---- phase-A constants; small ones first so the first matmul
            #      and first tanh wait on as few bytes as possible ----
            bh_sb = cp.tile([128, MT], F32)
            nc.sync.dma_start(bh_sb[:], bh[:])
            u_sb = cp.tile([128, MT], BF16)
            nc.sync.dma_start(u_sb[:], uu[:])
            id_sb = cp.tile([PB, PB], BF16)
            nc.sync.dma_start(id_sb[:], ident[:])
            wm_sb = []
            for m in range(MT):
                wm = cp.tile([128, KT, 128], BF16, name=f"wm{m}")
                nc.sync.dma_start(wm[:], whT8[m].rearrange("p (kt o) -> p kt o", o=128))
                wm_sb.append(wm)

            # masked alphaT tiles, built incrementally per batch
            am_sb = sp.tile([128, TT, PB, PB], BF16)
            nc.vector.memset(am_sb[:], 0.0)

            esum1 = sp.tile([1, PB], F32)
            einv1 = sp.tile([1, PB], F32)
            # r accumulates for all batches into one psum pair (masked
            # alphaT columns zero out the cross-batch garbage rows)
            r_ps = [pp.tile([PB, 512], F32, tag="ps", name=f"r_ps{i}") for i in range(2)]
            rn = 0

            # ---- phase A: per batch: big matmul, scores, softmax, alpha
            #      transpose into masked tiles, then that batch's r ----
            for b in range(PB):
                xc = xp.tile([128, KT, T], BF16)
                nc.gpsimd.dma_start(
                    xc[:],
                    xT[:, b * T : (b + 1) * T].rearrange("(kt p) n -> p kt n", p=128),
                )
                hb_t = hbp.tile([128, TT, H], BF16)
                nc.sync.dma_start(
                    hb_t[:],
                    hnat[b * T : (b + 1) * T, :].rearrange(
                        "(kt p) h -> p kt h", p=128
                    ),
                )
                tz_tiles = []
                for m in range(MT):
                    z_ps = pp.tile([128, T], F32, tag="ps")
                    for kt in range(KT):
                        nc.tensor.matmul(
                            z_ps[:],
                            wm_sb[m][:, kt, :],
                            xc[:, kt, :],
                            start=(kt == 0),
                            stop=(kt == KT - 1),
                        )
                    tz = tzp.tile([128, T], BF16)
                    nc.scalar.activation(tz[:], z_ps[:], TANH, bias=bh_sb[:, m : m + 1])
                    tz_tiles.append(tz)
                s_ps = pp.tile([1, T], F32, tag="ps")
                for m in range(MT):
                    nc.tensor.matmul(
                        s_ps[:1, :],
                        u_sb[:, m : m + 1],
                        tz_tiles[m][:],
                        start=(m == 0),
                        stop=(m == MT - 1),
                    )
                # softmax for this batch on partition 0
                sc_b = scp.tile([1, T], F32, tag="sc")
                nc.scalar.copy(sc_b[:1, :], s_ps[:1, :])
                e_b = scp.tile([1, T], F32, tag="eb")
                nc.scalar.activation(e_b[:1, :], sc_b[:1, :], EXP)
                nc.vector.reduce_sum(
                    esum1[:1, b : b + 1], e_b[:1, :], axis=mybir.AxisListType.X
                )
                nc.vector.reciprocal(einv1[:1, b : b + 1], esum1[:1, b : b + 1])
                a_b = scp.tile([1, T], BF16, tag="ab")
                nc.vector.tensor_scalar_mul(a_b[:1, :], e_b[:1, :], einv1[:1, b : b + 1])
                # transpose alpha_b into the masked [t, b] column
                for kt in range(TT):
                    t_ps = tpp.tile([128, PB], BF16, tag="tp")
                    nc.tensor.transpose(
                        t_ps[:, :1], a_b[:1, kt * 128 : (kt + 1) * 128], id_sb[:1, :1]
                    )
                    nc.scalar.copy(am_sb[:, kt, b, b : b + 1], t_ps[:, :1])
                # this batch's contribution to r
                for kt in range(TT):
                    for hc in range(2):
                        nc.tensor.matmul(
                            r_ps[hc][:],
                            am_sb[:, kt, b, :],
                            hb_t[:, kt, hc * 512 : (hc + 1) * 512],
                            start=(b == 0 and kt == 0),
                            stop=(b == PB - 1 and kt == TT - 1),
                        )
                        rn += 1

            # ---- late-loaded constants (sync queue, drain during phase A) ----
            wpT_sb = cp.tile([128, KT, H], BF16)
            nc.sync.dma_start(wpT_sb[:], wpT[:].rearrange("(kt p) n -> p kt n", p=128))
            wxh_sb = cp.tile([128, KT, H], BF16)
            nc.sync.dma_start(wxh_sb[:], wxh[:].rearrange("(kt p) n -> p kt n", p=128))
            wxl_sb = cp.tile([128, KT, H], BF16)
            nc.sync.dma_start(wxl_sb[:], wxl[:].rearrange("(kt p) n -> p kt n", p=128))
            hlh_sb = cp.tile([128, KT, B], BF16)
            nc.sync.dma_start(hlh_sb[:], hlh[:].rearrange("(kt p) j -> p kt j", p=128))
            hll_sb = cp.tile([128, KT, B], BF16)
            nc.sync.dma_start(hll_sb[:], hll[:].rearrange("(kt p) j -> p kt j", p=128))
            selA_sb = cp.tile([PB, 4, 128], BF16)
            nc.sync.dma_start(selA_sb[:], selA[:])
            bpx_sb = cp.tile([1, 2 * H], BF16)
            nc.sync.dma_start(bpx_sb[:], bpx[:])
            ones_sb = cp.tile([1, B], BF16)
            nc.sync.dma_start(ones_sb[:], ones[:])

            # ---- phase F: x2 = (hlast @ W_x.T + b_p + b_x) in split bf16 ----
            x2_sb = sp.tile([128, H], F32)
            for hc in range(2):
                x_ps = pp.tile([B, 512], F32, tag="ps")
                n = 0
                terms = [(hlh_sb, wxh_sb), (hll_sb, wxh_sb), (hlh_sb, wxl_sb)]
                nmm = len(terms) * KT + 2
                for lh, rh in terms:
                    for kt in range(KT):
                        nc.tensor.matmul(
                            x_ps[:],
                            lh[:, kt, :],
                            rh[:, kt, hc * 512 : (hc + 1) * 512],
                            start=(n == 0),
                            stop=(n == nmm - 1),
                        )
                        n += 1
                for row in range(2):
                    nc.tensor.matmul(
                        x_ps[:],
                        ones_sb[:1, :],
                        bpx_sb[:1, row * H + hc * 512 : row * H + (hc + 1) * 512],
                        start=(n == 0),
                        stop=(n == nmm - 1),
                    )
                    n += 1
                nc.scalar.copy(x2_sb[:B, hc * 512 : (hc + 1) * 512], x_ps[:])
                nc.scalar.copy(x2_sb[B:, hc * 512 : (hc + 1) * 512], x_ps[:])

            # ---- r -> rT -> p ----
            rflat_bf = sp.tile([PB, H], BF16)
            for hc in range(2):
                nc.scalar.copy(rflat_bf[:, hc * 512 : (hc + 1) * 512], r_ps[hc][:])
            rT_sb = sp.tile([128, KT, PB], BF16)
            for mt in range(KT):
                t_ps = tpp.tile([128, PB], BF16, tag="tp")
                nc.tensor.transpose(
                    t_ps[:], rflat_bf[:, mt * 128 : (mt + 1) * 128], id_sb[:]
                )
                nc.scalar.copy(rT_sb[:, mt, :], t_ps[:])
            p_sb = sp.tile([PB, H], BF16)
            for hc in range(2):
                p_ps = pp.tile([PB, 512], F32, tag="ps")
                for kt in range(KT):
                    nc.tensor.matmul(
                        p_ps[:],
                        rT_sb[:, kt, :],
                        wpT_sb[:, kt, hc * 512 : (hc + 1) * 512],
                        start=(kt == 0),
                        stop=(kt == KT - 1),
                    )
                nc.scalar.copy(p_sb[:, hc * 512 : (hc + 1) * 512], p_ps[:])

            # ---- phase G: out = tanh(A_sel @ p + x2) ----
            for q in range(4):
                for hc in range(2):
                    o_ps = pp.tile([128, 512], F32, tag="ps")
                    nc.tensor.matmul(
                        o_ps[:],
                        selA_sb[:, q, :],
                        p_sb[:, hc * 512 : (hc + 1) * 512],
                        start=True,
                        stop=True,
                    )
                    o_sb = op_.tile([128, 512], F32, tag="oadd")
                    nc.vector.tensor_add(
                        o_sb[:], o_ps[:], x2_sb[:, hc * 512 : (hc + 1) * 512]
                    )
                    o_sb2 = op_.tile([128, 512], F32, tag="otanh")
                    nc.scalar.activation(o_sb2[:], o_sb[:], TANH)
                    nc.sync.dma_start(
                        out[2 * q : 2 * q + 2, :, hc * 512 : (hc + 1) * 512].rearrange(
                            "i j h -> (i j) h"
                        ),
                        o_sb2[:],
                    )
    _split_excess_waits(nc)
    return nc


def _split_excess_waits(nc: bass.Bass, max_waits: int = 1) -> None:
    """Walrus's per-instruction sync-wait slots are limited; move excess
    on_wait entries onto wait-only NoOps inserted just before the
    instruction (same engine, so ordering is preserved)."""
    for fn in nc.m.functions:
        for blk in fn.blocks:
            new = []
            for inst in blk.instructions:
                si = inst.sync_info
                waits = list(si.on_wait) if si is not None and si.on_wait else []
                if len(waits) > max_waits:
                    extra, keep = waits[:-max_waits], waits[-max_waits:]
                    for ci in range(0, len(extra), max_waits):
                        nop = mybir.InstNoOp(
                            name=f"{inst.name}-wsplit{ci}", ins=[], outs=[]
                        )
                        nop.engine = inst.engine
                        nop.sync_info = mybir.SyncInfo(
                            on_wait=extra[ci : ci + max_waits], on_update=[]
                        )
                        new.append(nop)
                    inst.sync_info = mybir.SyncInfo(
                        on_wait=keep, on_update=list(si.on_update or [])
                    )
                new.append(inst)
            blk.instructions[:] = new


def _split_bf16(a: np.ndarray) -> tuple[np.ndarray, np.ndarray]:
    hi = a.astype(BF16_NP)
    lo = (a - hi.astype(np.float32)).astype(BF16_NP)
    return hi, lo


def _host_prep(inputs: dict) -> list[dict]:
    hidden = np.asarray(inputs["hidden"], np.float32)
    W_h = np.asarray(inputs["W_h"], np.float32)
    b_h = np.asarray(inputs["b_h"], np.float32)
    w_w = np.asarray(inputs["w_w"], np.float32)
    W_p = np.asarray(inputs["W_p"], np.float32)
    b_p = np.asarray(inputs["b_p"], np.float32)
    W_x = np.asarray(inputs["W_x"], np.float32)
    b_x = np.asarray(inputs["b_x"], np.float32)

    selA = np.zeros((PB, 4, 128), np.float32)
    for q in range(4):
        for m in range(128):
            selA[2 * q + m // 64, q, m] = 1.0

    wxT = np.ascontiguousarray(W_x.T)
    wx_hi, wx_lo = _split_bf16(wxT)
    hlT = np.ascontiguousarray(hidden[:, -1, :].T)
    hl_hi, hl_lo = _split_bf16(hlT)
    bpx_hi, bpx_lo = _split_bf16((b_p + b_x).reshape(1, H))

    shared = {
        "whT8": np.ascontiguousarray(
            W_h.T.reshape(KT, 128, MT, 128).transpose(2, 1, 0, 3).reshape(
                MT, 128, KT * 128
            )
        ).astype(BF16_NP),
        "bh": np.ascontiguousarray(b_h.reshape(MT, 128).T),
        "u": np.ascontiguousarray(w_w[0, :H].reshape(MT, 128).T).astype(BF16_NP),
        "wpT": np.ascontiguousarray(W_p.T).astype(BF16_NP),
        "wxT_hi": wx_hi,
        "wxT_lo": wx_lo,
        "hlastT_hi": hl_hi,
        "hlastT_lo": hl_lo,
        "selA": selA.astype(BF16_NP),
        "bpx": np.concatenate([bpx_hi, bpx_lo], axis=1),
        "ones": np.ones((1, B), BF16_NP),
        "ident": np.eye(PB, dtype=np.float32).astype(BF16_NP),
    }

    in_maps = []
    for c in range(NCORES):
        flat = hidden[c * PB : (c + 1) * PB].reshape(R, H)
        m = dict(shared)
        m["xT"] = np.ascontiguousarray(flat.T).astype(BF16_NP)
        m["hnat"] = flat.astype(BF16_NP)
        in_maps.append(m)
    return in_maps


def _ensure_ntff_hook() -> None:
    """The agent image's antenv lacks axon_hooks; register a shim module
    wired to the libaxon NTFF profile hook so trace=True works."""
    try:
        from antenv.axon_hooks import get_axon_ntff_profile_hook  # noqa: F401
        return
    except ImportError:
        pass
    import types
    import antenv
    from trn_agent_boot.trn_boot import _ntff_profile_via_ctypes

    mod = types.ModuleType("antenv.axon_hooks")
    holder = {"hook": _ntff_profile_via_ctypes("/opt/axon/libaxon_pjrt.so")}
    mod.get_axon_ntff_profile_hook = lambda: holder["hook"]
    mod.set_axon_ntff_profile_hook = lambda h: holder.__setitem__("hook", h)
    sys.modules["antenv.axon_hooks"] = mod
    antenv.axon_hooks = mod


def run(inputs: dict, trace: bool = False, **kw):
    if trace:
        _ensure_ntff_hook()
    if "nc" not in _CACHE:
        _CACHE["nc"] = _build_nc()
    nc = _CACHE["nc"]
    in_maps = _host_prep(inputs)
    res = run_bass_kernel_spmd(nc, in_maps, list(range(NCORES)), trace=trace, **kw)
    out = np.empty((B, B, H), np.float32)
    for c in range(NCORES):
        out[c * PB : (c + 1) * PB] = np.asarray(res.results[c]["out"], np.float32)
    return out, res


def kernel(**inputs) -> np.ndarray:
    out, _ = run(inputs)
    return out


# revision 20
# speedup vs baseline: 1.0619x; 1.0090x over previous
"""TRN2 Bass kernel for nn_Attention_76802605187492.

Math (B=64, T=512, H=1024, A=300):
  The aspect branch (aspect, W_v, b_v, w_w[:, H:], w_b) only adds a
  per-batch constant to the attention scores, which softmax cancels, so it
  does not affect the output at all.  What remains per batch b:
    scores[t] = u . tanh(W_h hidden[b,t] + b_h)      u = w_w[0, :H]
    alpha     = softmax_t(scores)
    r         = sum_t alpha[t] hidden[b,t]
    p_b       = r @ W_p.T
    x_j       = hidden[j,-1] @ W_x.T                  (all j)
    out[b,j]  = tanh(p_b + x_j + (b_p + b_x))         -> [B, B, H]

Sharding: data-parallel over batch across 8 cores (8 batches each). Each
core computes p for its batches, x for all 64 (tiny), and emits the
[8, 64, 1024] output slab.

All PE matmuls are bf16. The only output-critical matmul is the x term
(it dominates the pre-tanh activation), so it is computed in split
precision: x = hi@hi + lo@hi + hi@lo with hi/lo bf16 halves of the fp32
operands, accumulated in fp32 PSUM (error ~1e-5). b_p + b_x rides the
same accumulation via k=1 ones-matmuls, also in hi+lo halves.

Engine-AP partition bases must be 0/32/64(/96), so:
  - scores live on partition 0 as [1, 4096]; a SBUF->SBUF DMA reshapes
    them to [8, 512] (DMA has no partition-base restriction);
  - r for all 8 batches accumulates into ONE [8, 512] psum pair using
    per-batch column-masked alphaT tiles (garbage rows vanish because the
    masked columns are zero), so no per-row psum extraction is needed.

Final stage per output tile [128=(2 local-i x 64 j), 512]:
  psum = A_sel @ p   (A_sel constant 0/1 selector, k=8)
  out  = tanh(psum + x2)   with x2 = x duplicated on both partition halves
"""

import sys

sys.path.insert(0, "/opt/trn_rl_repo")
sys.path.insert(0, "/opt/trn_rl_repo/concourse")

import numpy as np
import ml_dtypes

import concourse.bass as bass
import concourse.mybir as mybir
from concourse import tile
from concourse.bass_utils import run_bass_kernel_spmd

F32 = mybir.dt.float32
BF16 = mybir.dt.bfloat16
BF16_NP = ml_dtypes.bfloat16
TANH = mybir.ActivationFunctionType.Tanh
EXP = mybir.ActivationFunctionType.Exp

B, T, H = 64, 512, 1024
NCORES = 8
PB = B // NCORES          # batches per core = 8
R = PB * T                # rows per core = 4096
KT = H // 128             # 8 k-tiles over h_in
MT = H // 128             # 8 m-tiles over h_out
TT = T // 128             # 4 t-tiles per batch

_CACHE: dict = {}


def _build_nc() -> bass.Bass:
    nc = bass.Bass()

    xT8 = nc.declare_dram_parameter("xT8", [PB, 128, KT * T], BF16, isOutput=False)
    hnat = nc.declare_dram_parameter("hnat", [R, H], BF16, isOutput=False)
    whT8 = nc.declare_dram_parameter("whT8", [MT, 128, KT * 128], BF16, isOutput=False)
    bh = nc.declare_dram_parameter("bh", [128, MT], F32, isOutput=False)
    uu = nc.declare_dram_parameter("u", [128, MT], BF16, isOutput=False)
    wpT = nc.declare_dram_parameter("wpT", [H, H], BF16, isOutput=False)
    wxh = nc.declare_dram_parameter("wxT_hi", [H, H], BF16, isOutput=False)
    wxl = nc.declare_dram_parameter("wxT_lo", [H, H], BF16, isOutput=False)
    hlh = nc.declare_dram_parameter("hlastT_hi", [H, B], BF16, isOutput=False)
    hll = nc.declare_dram_parameter("hlastT_lo", [H, B], BF16, isOutput=False)
    selA = nc.declare_dram_parameter("selA", [PB, 4, 128], BF16, isOutput=False)
    bpx = nc.declare_dram_parameter("bpx", [1, 2 * H], BF16, isOutput=False)
    ones = nc.declare_dram_parameter("ones", [1, B], BF16, isOutput=False)
    ident = nc.declare_dram_parameter("ident", [PB, PB], BF16, isOutput=False)
    out = nc.declare_dram_parameter("out", [PB, B, H], F32, isOutput=True)

    with tile.TileContext(nc) as tc:
        with (
            tc.tile_pool(name="const", bufs=1) as cp,
            tc.tile_pool(name="xchunk", bufs=2) as xp,
            tc.tile_pool(name="tz", bufs=10) as tzp,
            tc.tile_pool(name="hb", bufs=4) as hbp,
            tc.tile_pool(name="small", bufs=1) as sp,
            tc.tile_pool(name="sc", bufs=2) as scp,
            tc.tile_pool(name="outp", bufs=4) as op_,
            tc.tile_pool(name="ps", bufs=6, space=bass.MemorySpace.PSUM) as pp,
            tc.tile_pool(name="tps", bufs=2, space=bass.MemorySpace.PSUM) as tpp,
        ):
            # ---- phase-A constants; small ones first so the first matmul
            #      and first tanh wait on as few bytes as possible ----
            bh_sb = cp.tile([128, MT], F32)
            nc.sync.dma_start(bh_sb[:], bh[:])
            u_sb = cp.tile([128, MT], BF16)
            nc.sync.dma_start(u_sb[:], uu[:])
            id_sb = cp.tile([PB, PB], BF16)
            nc.sync.dma_start(id_sb[:], ident[:])
            wm_sb = []
            for m in range(MT):
                wm = cp.tile([128, KT, 128], BF16, name=f"wm{m}")
                nc.sync.dma_start(wm[:], whT8[m].rearrange("p (kt o) -> p kt o", o=128))
                wm_sb.append(wm)

            # masked alphaT tiles, built incrementally per batch
            am_sb = sp.tile([128, TT, PB, PB], BF16)
            nc.vector.memset(am_sb[:], 0.0)

            esum1 = sp.tile([1, PB], F32)
            einv1 = sp.tile([1, PB], F32)
            # r accumulates for all batches into one psum pair (masked
            # alphaT columns zero out the cross-batch garbage rows)
            r_ps = [pp.tile([PB, 512], F32, tag="ps", name=f"r_ps{i}") for i in range(2)]
            rn = 0

            # ---- phase A: per batch: big matmul, scores, softmax, alpha
            #      transpose into masked tiles, then that batch's r ----
            for b in range(PB):
                xc = xp.tile([128, KT, T], BF16)
                nc.gpsimd.dma_start(
                    xc[:], xT8[b].rearrange("p (kt n) -> p kt n", n=T)
                )
                hb_t = hbp.tile([128, TT, H], BF16)
                nc.sync.dma_start(
                    hb_t[:],
                    hnat[b * T : (b + 1) * T, :].rearrange(
                        "(kt p) h -> p kt h", p=128
                    ),
                )
                tz_tiles = []
                for m in range(MT):
                    z_ps = pp.tile([128, T], F32, tag="ps")
                    for kt in range(KT):
                        nc.tensor.matmul(
                            z_ps[:],
                            wm_sb[m][:, kt, :],
                            xc[:, kt, :],
                            start=(kt == 0),
                            stop=(kt == KT - 1),
                        )
                    tz = tzp.tile([128, T], BF16)
                    nc.scalar.activation(tz[:], z_ps[:], TANH, bias=bh_sb[:, m : m + 1])
                    tz_tiles.append(tz)
                s_ps = pp.tile([1, T], F32, tag="ps")
                for m in range(MT):
                    nc.tensor.matmul(
                        s_ps[:1, :],
                        u_sb[:, m : m + 1],
                        tz_tiles[m][:],
                        start=(m == 0),
                        stop=(m == MT - 1),
                    )
                # softmax for this batch on partition 0
                sc_b = scp.tile([1, T], F32, tag="sc")
                nc.scalar.copy(sc_b[:1, :], s_ps[:1, :])
                e_b = scp.tile([1, T], F32, tag="eb")
                nc.scalar.activation(e_b[:1, :], sc_b[:1, :], EXP)
                nc.vector.reduce_sum(
                    esum1[:1, b : b + 1], e_b[:1, :], axis=mybir.AxisListType.X
                )
                nc.vector.reciprocal(einv1[:1, b : b + 1], esum1[:1, b : b + 1])
                a_b = scp.tile([1, T], BF16, tag="ab")
                nc.vector.tensor_scalar_mul(a_b[:1, :], e_b[:1, :], einv1[:1, b : b + 1])
                # transpose alpha_b into the masked [t, b] column
                for kt in range(TT):
                    t_ps = tpp.tile([128, PB], BF16, tag="tp")
                    nc.tensor.transpose(
                        t_ps[:, :1], a_b[:1, kt * 128 : (kt + 1) * 128], id_sb[:1, :1]
                    )
                    nc.scalar.copy(am_sb[:, kt, b, b : b + 1], t_ps[:, :1])
                # this batch's contribution to r
                for kt in range(TT):
                    for hc in range(2):
                        nc.tensor.matmul(
                            r_ps[hc][:],
                            am_sb[:, kt, b, :],
                            hb_t[:, kt, hc * 512 : (hc + 1) * 512],
                            start=(b == 0 and kt == 0),
                            stop=(b == PB - 1 and kt == TT - 1),
                        )
                        rn += 1

            # ---- late-loaded constants (sync queue, drain during phase A) ----
            wpT_sb = cp.tile([128, KT, H], BF16)
            nc.sync.dma_start(wpT_sb[:], wpT[:].rearrange("(kt p) n -> p kt n", p=128))
            wxh_sb = cp.tile([128, KT, H], BF16)
            nc.sync.dma_start(wxh_sb[:], wxh[:].rearrange("(kt p) n -> p kt n", p=128))
            wxl_sb = cp.tile([128, KT, H], BF16)
            nc.sync.dma_start(wxl_sb[:], wxl[:].rearrange("(kt p) n -> p kt n", p=128))
            hlh_sb = cp.tile([128, KT, B], BF16)
            nc.sync.dma_start(hlh_sb[:], hlh[:].rearrange("(kt p) j -> p kt j", p=128))
            hll_sb = cp.tile([128, KT, B], BF16)
            nc.sync.dma_start(hll_sb[:], hll[:].rearrange("(kt p) j -> p kt j", p=128))
            selA_sb = cp.tile([PB, 4, 128], BF16)
            nc.sync.dma_start(selA_sb[:], selA[:])
            bpx_sb = cp.tile([1, 2 * H], BF16)
            nc.sync.dma_start(bpx_sb[:], bpx[:])
            ones_sb = cp.tile([1, B], BF16)
            nc.sync.dma_start(ones_sb[:], ones[:])

            # ---- phase F: x2 = (hlast @ W_x.T + b_p + b_x) in split bf16 ----
            x2_sb = sp.tile([128, H], F32)
            for hc in range(2):
                x_ps = pp.tile([B, 512], F32, tag="ps")
                n = 0
                terms = [(hlh_sb, wxh_sb), (hll_sb, wxh_sb), (hlh_sb, wxl_sb)]
                nmm = len(terms) * KT + 2
                for lh, rh in terms:
                    for kt in range(KT):
                        nc.tensor.matmul(
                            x_ps[:],
                            lh[:, kt, :],
                            rh[:, kt, hc * 512 : (hc + 1) * 512],
                            start=(n == 0),
                            stop=(n == nmm - 1),
                        )
                        n += 1
                for row in range(2):
                    nc.tensor.matmul(
                        x_ps[:],
                        ones_sb[:1, :],
                        bpx_sb[:1, row * H + hc * 512 : row * H + (hc + 1) * 512],
                        start=(n == 0),
                        stop=(n == nmm - 1),
                    )
                    n += 1
                nc.scalar.copy(x2_sb[:B, hc * 512 : (hc + 1) * 512], x_ps[:])
                nc.scalar.copy(x2_sb[B:, hc * 512 : (hc + 1) * 512], x_ps[:])

            # ---- r -> rT -> p ----
            rflat_bf = sp.tile([PB, H], BF16)
            for hc in range(2):
                nc.scalar.copy(rflat_bf[:, hc * 512 : (hc + 1) * 512], r_ps[hc][:])
            rT_sb = sp.tile([128, KT, PB], BF16)
            for mt in range(KT):
                t_ps = tpp.tile([128, PB], BF16, tag="tp")
                nc.tensor.transpose(
                    t_ps[:], rflat_bf[:, mt * 128 : (mt + 1) * 128], id_sb[:]
                )
                nc.scalar.copy(rT_sb[:, mt, :], t_ps[:])
            p_sb = sp.tile([PB, H], BF16)
            for hc in range(2):
                p_ps = pp.tile([PB, 512], F32, tag="ps")
                for kt in range(KT):
                    nc.tensor.matmul(
                        p_ps[:],
                        rT_sb[:, kt, :],
                        wpT_sb[:, kt, hc * 512 : (hc + 1) * 512],
                        start=(kt == 0),
                        stop=(kt == KT - 1),
                    )
                nc.scalar.copy(p_sb[:, hc * 512 : (hc + 1) * 512], p_ps[:])

            # ---- phase G: out = tanh(A_sel @ p + x2) ----
            for q in range(4):
                for hc in range(2):
                    o_ps = pp.tile([128, 512], F32, tag="ps")
                    nc.tensor.matmul(
                        o_ps[:],
                        selA_sb[:, q, :],
                        p_sb[:, hc * 512 : (hc + 1) * 512],
                        start=True,
                        stop=True,
                    )
                    o_sb = op_.tile([128, 512], F32, tag="oadd")
                    nc.vector.tensor_add(
                        o_sb[:], o_ps[:], x2_sb[:, hc * 512 : (hc + 1) * 512]
                    )
                    o_sb2 = op_.tile([128, 512], F32, tag="otanh")
                    nc.scalar.activation(o_sb2[:], o_sb[:], TANH)
                    nc.sync.dma_start(
                        out[2 * q : 2 * q + 2, :, hc * 512 : (hc + 1) * 512].rearrange(
                            "i j h -> (i j) h"
                        ),
                        o_sb2[:],
                    )
    _split_excess_waits(nc)
    return nc


def _split_excess_waits(nc: bass.Bass, max_waits: int = 1) -> None:
    """Walrus's per-instruction sync-wait slots are limited; move excess
    on_wait entries onto wait-only NoOps inserted just before the
    instruction (same engine, so ordering is preserved)."""
    for fn in nc.m.functions:
        for blk in fn.blocks:
            new = []
            for inst in blk.instructions:
                si = inst.sync_info
                waits = list(si.on_wait) if si is not None and si.on_wait else []
                if len(waits) > max_waits:
                    extra, keep = waits[:-max_waits], waits[-max_waits:]
                    for ci in range(0, len(extra), max_waits):
                        nop = mybir.InstNoOp(
                            name=f"{inst.name}-wsplit{ci}", ins=[], outs=[]
                        )
                        nop.engine = inst.engine
                        nop.sync_info = mybir.SyncInfo(
                            on_wait=extra[ci : ci + max_waits], on_update=[]
                        )
                        new.append(nop)
                    inst.sync_info = mybir.SyncInfo(
                        on_wait=keep, on_update=list(si.on_update or [])
                    )
                new.append(inst)
            blk.instructions[:] = new


def _split_bf16(a: np.ndarray) -> tuple[np.ndarray, np.ndarray]:
    hi = a.astype(BF16_NP)
    lo = (a - hi.astype(np.float32)).astype(BF16_NP)
    return hi, lo


def _host_prep(inputs: dict) -> list[dict]:
    hidden = np.asarray(inputs["hidden"], np.float32)
    W_h = np.asarray(inputs["W_h"], np.float32)
    b_h = np.asarray(inputs["b_h"], np.float32)
    w_w = np.asarray(inputs["w_w"], np.float32)
    W_p = np.asarray(inputs["W_p"], np.float32)
    b_p = np.asarray(inputs["b_p"], np.float32)
    W_x = np.asarray(inputs["W_x"], np.float32)
    b_x = np.asarray(inputs["b_x"], np.float32)

    selA = np.zeros((PB, 4, 128), np.float32)
    for q in range(4):
        for m in range(128):
            selA[2 * q + m // 64, q, m] = 1.0

    wxT = np.ascontiguousarray(W_x.T)
    wx_hi, wx_lo = _split_bf16(wxT)
    hlT = np.ascontiguousarray(hidden[:, -1, :].T)
    hl_hi, hl_lo = _split_bf16(hlT)
    bpx_hi, bpx_lo = _split_bf16((b_p + b_x).reshape(1, H))

    shared = {
        "whT8": np.ascontiguousarray(
            W_h.T.reshape(KT, 128, MT, 128).transpose(2, 1, 0, 3).reshape(
                MT, 128, KT * 128
            )
        ).astype(BF16_NP),
        "bh": np.ascontiguousarray(b_h.reshape(MT, 128).T),
        "u": np.ascontiguousarray(w_w[0, :H].reshape(MT, 128).T).astype(BF16_NP),
        "wpT": np.ascontiguousarray(W_p.T).astype(BF16_NP),
        "wxT_hi": wx_hi,
        "wxT_lo": wx_lo,
        "hlastT_hi": hl_hi,
        "hlastT_lo": hl_lo,
        "selA": selA.astype(BF16_NP),
        "bpx": np.concatenate([bpx_hi, bpx_lo], axis=1),
        "ones": np.ones((1, B), BF16_NP),
        "ident": np.eye(PB, dtype=np.float32).astype(BF16_NP),
    }

    in_maps = []
    for c in range(NCORES):
        flat = hidden[c * PB : (c + 1) * PB].reshape(R, H)
        m = dict(shared)
        m["xT8"] = np.ascontiguousarray(
            flat.reshape(PB, T, KT, 128).transpose(0, 3, 2, 1).reshape(
                PB, 128, KT * T
            )
        ).astype(BF16_NP)
        m["hnat"] = flat.astype(BF16_NP)
        in_maps.append(m)
    return in_maps


def _ensure_ntff_hook() -> None:
    """The agent image's antenv lacks axon_hooks; register a shim module
    wired to the libaxon NTFF profile hook so trace=True works."""
    try:
        from antenv.axon_hooks import get_axon_ntff_profile_hook  # noqa: F401
        return
    except ImportError:
        pass
    import types
    import antenv
    from trn_agent_boot.trn_boot import _ntff_profile_via_ctypes

    mod = types.ModuleType("antenv.axon_hooks")
    holder = {"hook": _ntff_profile_via_ctypes("/opt/axon/libaxon_pjrt.so")}
    mod.get_axon_ntff_profile_hook = lambda: holder["hook"]
    mod.set_axon_ntff_profile_hook = lambda h: holder.__setitem__("hook", h)
    sys.modules["antenv.axon_hooks"] = mod
    antenv.axon_hooks = mod


def run(inputs: dict, trace: bool = False, **kw):
    if trace:
        _ensure_ntff_hook()
    if "nc" not in _CACHE:
        _CACHE["nc"] = _build_nc()
    nc = _CACHE["nc"]
    in_maps = _host_prep(inputs)
    res = run_bass_kernel_spmd(nc, in_maps, list(range(NCORES)), trace=trace, **kw)
    out = np.empty((B, B, H), np.float32)
    for c in range(NCORES):
        out[c * PB : (c + 1) * PB] = np.asarray(res.results[c]["out"], np.float32)
    return out, res


def kernel(**inputs) -> np.ndarray:
    out, _ = run(inputs)
    return out


# revision 21
# speedup vs baseline: 1.5380x; 1.4483x over previous
"""TRN2 Bass kernel for nn_Attention_76802605187492.

Math (B=64, T=512, H=1024, A=300):
  The aspect branch (aspect, W_v, b_v, w_w[:, H:], w_b) only adds a
  per-batch constant to the attention scores, which softmax cancels, so it
  does not affect the output at all.  What remains per batch b:
    scores[t] = u . tanh(W_h hidden[b,t] + b_h)      u = w_w[0, :H]
    alpha     = softmax_t(scores)
    r         = sum_t alpha[t] hidden[b,t]
    p_b       = r @ W_p.T
    x_j       = hidden[j,-1] @ W_x.T                  (all j)
    out[b,j]  = tanh(p_b + x_j + (b_p + b_x))         -> [B, B, H]

Sharding: data-parallel over batch across 8 cores (8 batches each). Each
core computes p for its batches, x for all 64 (tiny), and emits the
[8, 64, 1024] output slab.

All PE matmuls are bf16. The only output-critical matmul is the x term
(it dominates the pre-tanh activation), so it is computed in split
precision: x = hi@hi + lo@hi + hi@lo with hi/lo bf16 halves of the fp32
operands, accumulated in fp32 PSUM (error ~1e-5). b_p + b_x rides the
same accumulation via k=1 ones-matmuls, also in hi+lo halves.

Engine-AP partition bases must be 0/32/64(/96), so:
  - scores live on partition 0 as [1, 4096]; a SBUF->SBUF DMA reshapes
    them to [8, 512] (DMA has no partition-base restriction);
  - r for all 8 batches accumulates into ONE [8, 512] psum pair using
    per-batch column-masked alphaT tiles (garbage rows vanish because the
    masked columns are zero), so no per-row psum extraction is needed.

Final stage per output tile [128=(2 local-i x 64 j), 512]:
  psum = A_sel @ p   (A_sel constant 0/1 selector, k=8)
  out  = tanh(psum + x2)   with x2 = x duplicated on both partition halves
"""

import os
import sys

sys.path.insert(0, "/opt/trn_rl_repo")
sys.path.insert(0, "/opt/trn_rl_repo/concourse")

import numpy as np
import ml_dtypes

import concourse.bass as bass
import concourse.mybir as mybir
from concourse import tile
from concourse.bass_utils import run_bass_kernel_spmd

F32 = mybir.dt.float32
BF16 = mybir.dt.bfloat16
BF16_NP = ml_dtypes.bfloat16
TANH = mybir.ActivationFunctionType.Tanh
EXP = mybir.ActivationFunctionType.Exp
FP8 = mybir.dt.float8e4
FP8_NP = ml_dtypes.float8_e4m3
FP8_BIG = os.environ.get("KFP8", "0") == "1"
WSCALE = 16.0

B, T, H = 64, 512, 1024
NCORES = 8
PB = B // NCORES          # batches per core = 8
R = PB * T                # rows per core = 4096
KT = H // 128             # 8 k-tiles over h_in
MT = H // 128             # 8 m-tiles over h_out
TT = T // 128             # 4 t-tiles per batch
KT2 = H // 256            # 4 double-row k-tiles (fp8 path)

_CACHE: dict = {}


def _build_nc() -> bass.Bass:
    nc = bass.Bass()

    if FP8_BIG:
        xQ8 = nc.declare_dram_parameter(
            "xQ8", [PB, 128, KT2 * 2 * T], FP8, isOutput=False
        )
        whQ8 = nc.declare_dram_parameter(
            "whQ8", [MT, 128, KT2 * 2 * 128], FP8, isOutput=False
        )
    else:
        xT8 = nc.declare_dram_parameter(
            "xT8", [PB, 128, KT * T], BF16, isOutput=False
        )
        whT8 = nc.declare_dram_parameter(
            "whT8", [MT, 128, KT * 128], BF16, isOutput=False
        )
    hnat = nc.declare_dram_parameter("hnat", [R, H], BF16, isOutput=False)
    bh = nc.declare_dram_parameter("bh", [128, MT], F32, isOutput=False)
    uu = nc.declare_dram_parameter("u", [128, MT], BF16, isOutput=False)
    wpT = nc.declare_dram_parameter("wpT", [H, H], BF16, isOutput=False)
    wxh = nc.declare_dram_parameter("wxT_hi", [H, H], BF16, isOutput=False)
    wxl = nc.declare_dram_parameter("wxT_lo", [H, H], BF16, isOutput=False)
    hlh = nc.declare_dram_parameter("hlastT_hi", [H, B], BF16, isOutput=False)
    hll = nc.declare_dram_parameter("hlastT_lo", [H, B], BF16, isOutput=False)
    selA = nc.declare_dram_parameter("selA", [PB, 4, 128], BF16, isOutput=False)
    bpx = nc.declare_dram_parameter("bpx", [1, 2 * H], BF16, isOutput=False)
    ones = nc.declare_dram_parameter("ones", [1, B], BF16, isOutput=False)
    ident = nc.declare_dram_parameter("ident", [PB, PB], BF16, isOutput=False)
    out = nc.declare_dram_parameter("out", [PB, B, H], F32, isOutput=True)

    with tile.TileContext(nc) as tc:
        with (
            tc.tile_pool(name="const", bufs=1) as cp,
            tc.tile_pool(name="xchunk", bufs=2) as xp,
            tc.tile_pool(name="tz", bufs=10) as tzp,
            tc.tile_pool(name="hb", bufs=2) as hbp,
            tc.tile_pool(name="small", bufs=1) as sp,
            tc.tile_pool(name="sc", bufs=2) as scp,
            tc.tile_pool(name="outp", bufs=4) as op_,
            tc.tile_pool(name="ps", bufs=6, space=bass.MemorySpace.PSUM) as pp,
            tc.tile_pool(name="tps", bufs=2, space=bass.MemorySpace.PSUM) as tpp,
        ):
            # ---- phase-A constants; small ones first so the first matmul
            #      and first tanh wait on as few bytes as possible ----
            bh_sb = cp.tile([128, MT], F32)
            nc.sync.dma_start(bh_sb[:], bh[:])
            u_sb = cp.tile([128, MT], BF16)
            nc.sync.dma_start(u_sb[:], uu[:])
            id_sb = cp.tile([PB, PB], BF16)
            nc.sync.dma_start(id_sb[:], ident[:])
            wm_sb = []
            for m in range(MT):
                if FP8_BIG:
                    wm = cp.tile([128, KT2, 2, 128], FP8, name=f"wm{m}")
                    nc.sync.dma_start(
                        wm[:],
                        whQ8[m].rearrange("p (kt j o) -> p kt j o", j=2, o=128),
                    )
                else:
                    wm = cp.tile([128, KT, 128], BF16, name=f"wm{m}")
                    nc.sync.dma_start(
                        wm[:], whT8[m].rearrange("p (kt o) -> p kt o", o=128)
                    )
                wm_sb.append(wm)

            # masked alphaT tiles, built incrementally per batch
            am_sb = sp.tile([128, TT, PB, PB], BF16)
            nc.vector.memset(am_sb[:], 0.0)

            esum1 = sp.tile([1, PB], F32)
            einv1 = sp.tile([1, PB], F32)
            # r accumulates for all batches into one psum pair (masked
            # alphaT columns zero out the cross-batch garbage rows)
            r_ps = [pp.tile([PB, 512], F32, tag="ps", name=f"r_ps{i}") for i in range(2)]
            rn = 0

            # ---- phase A: per batch: big matmul, scores, softmax, alpha
            #      transpose into masked tiles, then that batch's r ----
            def emit_r(bb, hb_t):
                for kt in range(TT):
                    for hc in range(2):
                        nc.tensor.matmul(
                            r_ps[hc][:],
                            am_sb[:, kt, bb, :],
                            hb_t[:, kt, hc * 512 : (hc + 1) * 512],
                            start=(bb == 0 and kt == 0),
                            stop=(bb == PB - 1 and kt == TT - 1),
                        )

            prev = None
            for b in range(PB):
                if FP8_BIG:
                    xc = xp.tile([128, KT2, 2, T], FP8)
                    nc.gpsimd.dma_start(
                        xc[:], xQ8[b].rearrange("p (kt j n) -> p kt j n", j=2, n=T)
                    )
                else:
                    xc = xp.tile([128, KT, T], BF16)
                    nc.gpsimd.dma_start(
                        xc[:], xT8[b].rearrange("p (kt n) -> p kt n", n=T)
                    )
                tz_tiles = []
                for m in range(MT):
                    z_ps = pp.tile([128, T], F32, tag="ps")
                    if FP8_BIG:
                        for kt in range(KT2):
                            nc.tensor.matmul(
                                z_ps[:],
                                wm_sb[m][:, kt, :, :],
                                xc[:, kt, :, :],
                                start=(kt == 0),
                                stop=(kt == KT2 - 1),
                                perf_mode=mybir.MatmulPerfMode.DoubleRow,
                            )
                    else:
                        for kt in range(KT):
                            nc.tensor.matmul(
                                z_ps[:],
                                wm_sb[m][:, kt, :],
                                xc[:, kt, :],
                                start=(kt == 0),
                                stop=(kt == KT - 1),
                            )
                    tz = tzp.tile([128, T], BF16)
                    nc.scalar.activation(
                        tz[:],
                        z_ps[:],
                        TANH,
                        bias=bh_sb[:, m : m + 1],
                        scale=(1.0 / WSCALE) if FP8_BIG else 1.0,
                    )
                    tz_tiles.append(tz)
                s_ps = pp.tile([1, T], F32, tag="ps")
                for m in range(MT):
                    nc.tensor.matmul(
                        s_ps[:1, :],
                        u_sb[:, m : m + 1],
                        tz_tiles[m][:],
                        start=(m == 0),
                        stop=(m == MT - 1),
                    )
                # softmax for this batch on partition 0
                sc_b = scp.tile([1, T], F32, tag="sc")
                nc.scalar.copy(sc_b[:1, :], s_ps[:1, :])
                e_b = scp.tile([1, T], F32, tag="eb")
                nc.scalar.activation(e_b[:1, :], sc_b[:1, :], EXP)
                nc.vector.reduce_sum(
                    esum1[:1, b : b + 1], e_b[:1, :], axis=mybir.AxisListType.X
                )
                nc.vector.reciprocal(einv1[:1, b : b + 1], esum1[:1, b : b + 1])
                a_b = scp.tile([1, T], BF16, tag="ab")
                nc.vector.tensor_scalar_mul(a_b[:1, :], e_b[:1, :], einv1[:1, b : b + 1])
                # transpose alpha_b into the masked [t, b] column
                for kt in range(TT):
                    t_ps = tpp.tile([128, PB], BF16, tag="tp")
                    nc.tensor.transpose(
                        t_ps[:, :1], a_b[:1, kt * 128 : (kt + 1) * 128], id_sb[:1, :1]
                    )
                    nc.scalar.copy(am_sb[:, kt, b, b : b + 1], t_ps[:, :1])
                # previous batch's r (its hidden had a full chunk to arrive)
                if prev is not None:
                    emit_r(*prev)
                hb_t = hbp.tile([128, TT, H], BF16)
                nc.sync.dma_start(
                    hb_t[:],
                    hnat[b * T : (b + 1) * T, :].rearrange(
                        "(kt p) h -> p kt h", p=128
                    ),
                )
                prev = (b, hb_t)
            emit_r(*prev)

            # ---- late-loaded constants (sync queue, drain during phase A) ----
            wpT_sb = cp.tile([128, KT, H], BF16)
            nc.sync.dma_start(wpT_sb[:], wpT[:].rearrange("(kt p) n -> p kt n", p=128))
            wxh_sb = cp.tile([128, KT, H], BF16)
            nc.sync.dma_start(wxh_sb[:], wxh[:].rearrange("(kt p) n -> p kt n", p=128))
            wxl_sb = cp.tile([128, KT, H], BF16)
            nc.sync.dma_start(wxl_sb[:], wxl[:].rearrange("(kt p) n -> p kt n", p=128))
            hlh_sb = cp.tile([128, KT, B], BF16)
            nc.sync.dma_start(hlh_sb[:], hlh[:].rearrange("(kt p) j -> p kt j", p=128))
            hll_sb = cp.tile([128, KT, B], BF16)
            nc.sync.dma_start(hll_sb[:], hll[:].rearrange("(kt p) j -> p kt j", p=128))
            selA_sb = cp.tile([PB, 4, 128], BF16)
            nc.sync.dma_start(selA_sb[:], selA[:])
            bpx_sb = cp.tile([1, 2 * H], BF16)
            nc.sync.dma_start(bpx_sb[:], bpx[:])
            ones_sb = cp.tile([1, B], BF16)
            nc.sync.dma_start(ones_sb[:], ones[:])

            # ---- phase F: x2 = (hlast @ W_x.T + b_p + b_x) in split bf16 ----
            x2_sb = sp.tile([128, H], F32)
            for hc in range(2):
                x_ps = pp.tile([B, 512], F32, tag="ps")
                n = 0
                terms = [(hlh_sb, wxh_sb), (hll_sb, wxh_sb), (hlh_sb, wxl_sb)]
                nmm = len(terms) * KT + 2
                for lh, rh in terms:
                    for kt in range(KT):
                        nc.tensor.matmul(
                            x_ps[:],
                            lh[:, kt, :],
                            rh[:, kt, hc * 512 : (hc + 1) * 512],
                            start=(n == 0),
                            stop=(n == nmm - 1),
                        )
                        n += 1
                for row in range(2):
                    nc.tensor.matmul(
                        x_ps[:],
                        ones_sb[:1, :],
                        bpx_sb[:1, row * H + hc * 512 : row * H + (hc + 1) * 512],
                        start=(n == 0),
                        stop=(n == nmm - 1),
                    )
                    n += 1
                nc.scalar.copy(x2_sb[:B, hc * 512 : (hc + 1) * 512], x_ps[:])
                nc.scalar.copy(x2_sb[B:, hc * 512 : (hc + 1) * 512], x_ps[:])

            # ---- r -> rT -> p ----
            rflat_bf = sp.tile([PB, H], BF16)
            for hc in range(2):
                nc.scalar.copy(rflat_bf[:, hc * 512 : (hc + 1) * 512], r_ps[hc][:])
            rT_sb = sp.tile([128, KT, PB], BF16)
            for mt in range(KT):
                t_ps = tpp.tile([128, PB], BF16, tag="tp")
                nc.tensor.transpose(
                    t_ps[:], rflat_bf[:, mt * 128 : (mt + 1) * 128], id_sb[:]
                )
                nc.scalar.copy(rT_sb[:, mt, :], t_ps[:])
            p_sb = sp.tile([PB, H], BF16)
            for hc in range(2):
                p_ps = pp.tile([PB, 512], F32, tag="ps")
                for kt in range(KT):
                    nc.tensor.matmul(
                        p_ps[:],
                        rT_sb[:, kt, :],
                        wpT_sb[:, kt, hc * 512 : (hc + 1) * 512],
                        start=(kt == 0),
                        stop=(kt == KT - 1),
                    )
                nc.scalar.copy(p_sb[:, hc * 512 : (hc + 1) * 512], p_ps[:])

            # ---- phase G: out = tanh(A_sel @ p + x2) ----
            for q in range(4):
                for hc in range(2):
                    o_ps = pp.tile([128, 512], F32, tag="ps")
                    nc.tensor.matmul(
                        o_ps[:],
                        selA_sb[:, q, :],
                        p_sb[:, hc * 512 : (hc + 1) * 512],
                        start=True,
                        stop=True,
                    )
                    o_sb = op_.tile([128, 512], F32, tag="oadd")
                    nc.vector.tensor_add(
                        o_sb[:], o_ps[:], x2_sb[:, hc * 512 : (hc + 1) * 512]
                    )
                    o_sb2 = op_.tile([128, 512], F32, tag="otanh")
                    nc.scalar.activation(o_sb2[:], o_sb[:], TANH)
                    nc.sync.dma_start(
                        out[2 * q : 2 * q + 2, :, hc * 512 : (hc + 1) * 512].rearrange(
                            "i j h -> (i j) h"
                        ),
                        o_sb2[:],
                    )
    _split_excess_waits(nc)
    return nc


def _split_excess_waits(nc: bass.Bass, max_waits: int = 1) -> None:
    """Walrus's per-instruction sync-wait slots are limited; move excess
    on_wait entries onto wait-only NoOps inserted just before the
    instruction (same engine, so ordering is preserved)."""
    for fn in nc.m.functions:
        for blk in fn.blocks:
            new = []
            for inst in blk.instructions:
                si = inst.sync_info
                waits = list(si.on_wait) if si is not None and si.on_wait else []
                if len(waits) > max_waits:
                    extra, keep = waits[:-max_waits], waits[-max_waits:]
                    for ci in range(0, len(extra), max_waits):
                        nop = mybir.InstNoOp(
                            name=f"{inst.name}-wsplit{ci}", ins=[], outs=[]
                        )
                        nop.engine = inst.engine
                        nop.sync_info = mybir.SyncInfo(
                            on_wait=extra[ci : ci + max_waits], on_update=[]
                        )
                        new.append(nop)
                    inst.sync_info = mybir.SyncInfo(
                        on_wait=keep, on_update=list(si.on_update or [])
                    )
                new.append(inst)
            blk.instructions[:] = new


def _split_bf16(a: np.ndarray) -> tuple[np.ndarray, np.ndarray]:
    hi = a.astype(BF16_NP)
    lo = (a - hi.astype(np.float32)).astype(BF16_NP)
    return hi, lo


def _host_prep(inputs: dict) -> list[dict]:
    hidden = np.asarray(inputs["hidden"], np.float32)
    W_h = np.asarray(inputs["W_h"], np.float32)
    b_h = np.asarray(inputs["b_h"], np.float32)
    w_w = np.asarray(inputs["w_w"], np.float32)
    W_p = np.asarray(inputs["W_p"], np.float32)
    b_p = np.asarray(inputs["b_p"], np.float32)
    W_x = np.asarray(inputs["W_x"], np.float32)
    b_x = np.asarray(inputs["b_x"], np.float32)

    selA = np.zeros((PB, 4, 128), np.float32)
    for q in range(4):
        for m in range(128):
            selA[2 * q + m // 64, q, m] = 1.0

    wxT = np.ascontiguousarray(W_x.T)
    wx_hi, wx_lo = _split_bf16(wxT)
    hlT = np.ascontiguousarray(hidden[:, -1, :].T)
    hl_hi, hl_lo = _split_bf16(hlT)
    bpx_hi, bpx_lo = _split_bf16((b_p + b_x).reshape(1, H))

    shared = {}
    if FP8_BIG:
        shared["whQ8"] = np.ascontiguousarray(
            (W_h.T * WSCALE)
            .reshape(KT2, 128, 2, MT, 128)
            .transpose(3, 1, 0, 2, 4)
            .reshape(MT, 128, KT2 * 2 * 128)
        ).astype(FP8_NP)
    else:
        shared["whT8"] = np.ascontiguousarray(
            W_h.T.reshape(KT, 128, MT, 128).transpose(2, 1, 0, 3).reshape(
                MT, 128, KT * 128
            )
        ).astype(BF16_NP)
    shared.update({

        "bh": np.ascontiguousarray(b_h.reshape(MT, 128).T),
        "u": np.ascontiguousarray(w_w[0, :H].reshape(MT, 128).T).astype(BF16_NP),
        "wpT": np.ascontiguousarray(W_p.T).astype(BF16_NP),
        "wxT_hi": wx_hi,
        "wxT_lo": wx_lo,
        "hlastT_hi": hl_hi,
        "hlastT_lo": hl_lo,
        "selA": selA.astype(BF16_NP),
        "bpx": np.concatenate([bpx_hi, bpx_lo], axis=1),
        "ones": np.ones((1, B), BF16_NP),
        "ident": np.eye(PB, dtype=np.float32).astype(BF16_NP),
    })

    in_maps = []
    for c in range(NCORES):
        flat = hidden[c * PB : (c + 1) * PB].reshape(R, H)
        m = dict(shared)
        if FP8_BIG:
            m["xQ8"] = np.ascontiguousarray(
                flat.reshape(PB, T, KT2, 128, 2)
                .transpose(0, 3, 2, 4, 1)
                .reshape(PB, 128, KT2 * 2 * T)
            ).astype(FP8_NP)
        else:
            m["xT8"] = np.ascontiguousarray(
                flat.reshape(PB, T, KT, 128).transpose(0, 3, 2, 1).reshape(
                    PB, 128, KT * T
                )
            ).astype(BF16_NP)
        m["hnat"] = flat.astype(BF16_NP)
        in_maps.append(m)
    return in_maps


def _ensure_ntff_hook() -> None:
    """The agent image's antenv lacks axon_hooks; register a shim module
    wired to the libaxon NTFF profile hook so trace=True works."""
    try:
        from antenv.axon_hooks import get_axon_ntff_profile_hook  # noqa: F401
        return
    except ImportError:
        pass
    import types
    import antenv
    from trn_agent_boot.trn_boot import _ntff_profile_via_ctypes

    mod = types.ModuleType("antenv.axon_hooks")
    holder = {"hook": _ntff_profile_via_ctypes("/opt/axon/libaxon_pjrt.so")}
    mod.get_axon_ntff_profile_hook = lambda: holder["hook"]
    mod.set_axon_ntff_profile_hook = lambda h: holder.__setitem__("hook", h)
    sys.modules["antenv.axon_hooks"] = mod
    antenv.axon_hooks = mod


def run(inputs: dict, trace: bool = False, **kw):
    if trace:
        _ensure_ntff_hook()
    if "nc" not in _CACHE:
        _CACHE["nc"] = _build_nc()
    nc = _CACHE["nc"]
    in_maps = _host_prep(inputs)
    res = run_bass_kernel_spmd(nc, in_maps, list(range(NCORES)), trace=trace, **kw)
    out = np.empty((B, B, H), np.float32)
    for c in range(NCORES):
        out[c * PB : (c + 1) * PB] = np.asarray(res.results[c]["out"], np.float32)
    return out, res


def kernel(**inputs) -> np.ndarray:
    out, _ = run(inputs)
    return out


# revision 22
# speedup vs baseline: 1.6022x; 1.0417x over previous
"""TRN2 Bass kernel for nn_Attention_76802605187492.

Math (B=64, T=512, H=1024, A=300):
  The aspect branch (aspect, W_v, b_v, w_w[:, H:], w_b) only adds a
  per-batch constant to the attention scores, which softmax cancels, so it
  does not affect the output at all.  What remains per batch b:
    scores[t] = u . tanh(W_h hidden[b,t] + b_h)      u = w_w[0, :H]
    alpha     = softmax_t(scores)
    r         = sum_t alpha[t] hidden[b,t]
    p_b       = r @ W_p.T
    x_j       = hidden[j,-1] @ W_x.T                  (all j)
    out[b,j]  = tanh(p_b + x_j + (b_p + b_x))         -> [B, B, H]

Sharding: data-parallel over batch across 8 cores (8 batches each). Each
core computes p for its batches, x for all 64 (tiny), and emits the
[8, 64, 1024] output slab.

All PE matmuls are bf16. The only output-critical matmul is the x term
(it dominates the pre-tanh activation), so it is computed in split
precision: x = hi@hi + lo@hi + hi@lo with hi/lo bf16 halves of the fp32
operands, accumulated in fp32 PSUM (error ~1e-5). b_p + b_x rides the
same accumulation via k=1 ones-matmuls, also in hi+lo halves.

Engine-AP partition bases must be 0/32/64(/96), so:
  - scores live on partition 0 as [1, 4096]; a SBUF->SBUF DMA reshapes
    them to [8, 512] (DMA has no partition-base restriction);
  - r for all 8 batches accumulates into ONE [8, 512] psum pair using
    per-batch column-masked alphaT tiles (garbage rows vanish because the
    masked columns are zero), so no per-row psum extraction is needed.

Final stage per output tile [128=(2 local-i x 64 j), 512]:
  psum = A_sel @ p   (A_sel constant 0/1 selector, k=8)
  out  = tanh(psum + x2)   with x2 = x duplicated on both partition halves
"""

import os
import sys

sys.path.insert(0, "/opt/trn_rl_repo")
sys.path.insert(0, "/opt/trn_rl_repo/concourse")

import numpy as np
import ml_dtypes

import concourse.bass as bass
import concourse.mybir as mybir
from concourse import tile
from concourse.bass_utils import run_bass_kernel_spmd

F32 = mybir.dt.float32
BF16 = mybir.dt.bfloat16
BF16_NP = ml_dtypes.bfloat16
TANH = mybir.ActivationFunctionType.Tanh
EXP = mybir.ActivationFunctionType.Exp
FP8 = mybir.dt.float8e4
FP8_NP = ml_dtypes.float8_e4m3
FP8_BIG = os.environ.get("KFP8", "0") == "1"
WSCALE = 16.0

B, T, H = 64, 512, 1024
NCORES = 8
PB = B // NCORES          # batches per core = 8
R = PB * T                # rows per core = 4096
KT = H // 128             # 8 k-tiles over h_in
MT = H // 128             # 8 m-tiles over h_out
TT = T // 128             # 4 t-tiles per batch
KT2 = H // 256            # 4 double-row k-tiles (fp8 path)
TT2 = T // 256            # 2 double-row t-tiles (fp8 r path)
ASCALE = 256.0            # alpha pre-scale so fp8 stays in normal range

_CACHE: dict = {}


def _build_nc() -> bass.Bass:
    nc = bass.Bass()

    if FP8_BIG:
        xQ8 = nc.declare_dram_parameter(
            "xQ8", [PB, 128, KT2 * 2 * T], FP8, isOutput=False
        )
        whQ8 = nc.declare_dram_parameter(
            "whQ8", [MT, 128, KT2 * 2 * 128], FP8, isOutput=False
        )
    else:
        xT8 = nc.declare_dram_parameter(
            "xT8", [PB, 128, KT * T], BF16, isOutput=False
        )
        whT8 = nc.declare_dram_parameter(
            "whT8", [MT, 128, KT * 128], BF16, isOutput=False
        )
    hQ8 = nc.declare_dram_parameter("hQ8", [PB, 128, TT2 * 2 * H], FP8, isOutput=False)
    bh = nc.declare_dram_parameter("bh", [128, MT], F32, isOutput=False)
    uu = nc.declare_dram_parameter("u", [128, MT], BF16, isOutput=False)
    wpT = nc.declare_dram_parameter("wpT", [H, H], BF16, isOutput=False)
    wxh = nc.declare_dram_parameter("wxT_hi", [H, H], BF16, isOutput=False)
    wxl = nc.declare_dram_parameter("wxT_lo", [H, H], BF16, isOutput=False)
    hlh = nc.declare_dram_parameter("hlastT_hi", [H, B], BF16, isOutput=False)
    hll = nc.declare_dram_parameter("hlastT_lo", [H, B], BF16, isOutput=False)
    selA = nc.declare_dram_parameter("selA", [PB, 4, 128], BF16, isOutput=False)
    bpx = nc.declare_dram_parameter("bpx", [1, 2 * H], BF16, isOutput=False)
    ones = nc.declare_dram_parameter("ones", [1, B], BF16, isOutput=False)
    ident = nc.declare_dram_parameter("ident", [PB, PB], BF16, isOutput=False)
    out = nc.declare_dram_parameter("out", [PB, B, H], F32, isOutput=True)

    with tile.TileContext(nc) as tc:
        with (
            tc.tile_pool(name="const", bufs=1) as cp,
            tc.tile_pool(name="xchunk", bufs=2) as xp,
            tc.tile_pool(name="tz", bufs=10) as tzp,
            tc.tile_pool(name="hb", bufs=2) as hbp,
            tc.tile_pool(name="small", bufs=1) as sp,
            tc.tile_pool(name="sc", bufs=2) as scp,
            tc.tile_pool(name="outp", bufs=4) as op_,
            tc.tile_pool(name="ps", bufs=6, space=bass.MemorySpace.PSUM) as pp,
            tc.tile_pool(name="tps", bufs=2, space=bass.MemorySpace.PSUM) as tpp,
        ):
            # ---- phase-A constants; small ones first so the first matmul
            #      and first tanh wait on as few bytes as possible ----
            bh_sb = cp.tile([128, MT], F32)
            nc.sync.dma_start(bh_sb[:], bh[:])
            u_sb = cp.tile([128, MT], BF16)
            nc.sync.dma_start(u_sb[:], uu[:])
            id_sb = cp.tile([PB, PB], BF16)
            nc.sync.dma_start(id_sb[:], ident[:])
            wm_sb = []
            for m in range(MT):
                if FP8_BIG:
                    wm = cp.tile([128, KT2, 2, 128], FP8, name=f"wm{m}")
                    nc.sync.dma_start(
                        wm[:],
                        whQ8[m].rearrange("p (kt j o) -> p kt j o", j=2, o=128),
                    )
                else:
                    wm = cp.tile([128, KT, 128], BF16, name=f"wm{m}")
                    nc.sync.dma_start(
                        wm[:], whT8[m].rearrange("p (kt o) -> p kt o", o=128)
                    )
                wm_sb.append(wm)

            # masked alphaT tiles, built incrementally per batch
            am_sb = sp.tile([128, TT2, 2, PB, PB], FP8)
            nc.vector.memset(am_sb[:], 0.0)

            esum1 = sp.tile([1, PB], F32)
            einv1 = sp.tile([1, PB], F32)
            # r accumulates for all batches into one psum pair (masked
            # alphaT columns zero out the cross-batch garbage rows)
            r_ps = [pp.tile([PB, 512], F32, tag="ps", name=f"r_ps{i}") for i in range(2)]
            rn = 0

            # ---- phase A: per batch: big matmul, scores, softmax, alpha
            #      transpose into masked tiles, then that batch's r ----
            def emit_r(bb, hb_t):
                for kt in range(TT2):
                    for hc in range(2):
                        nc.tensor.matmul(
                            r_ps[hc][:],
                            am_sb[:, kt, :, bb, :],
                            hb_t[:, kt, :, hc * 512 : (hc + 1) * 512],
                            start=(bb == 0 and kt == 0),
                            stop=(bb == PB - 1 and kt == TT2 - 1),
                            perf_mode=mybir.MatmulPerfMode.DoubleRow,
                        )

            prev = None
            for b in range(PB):
                if FP8_BIG:
                    xc = xp.tile([128, KT2, 2, T], FP8)
                    nc.gpsimd.dma_start(
                        xc[:], xQ8[b].rearrange("p (kt j n) -> p kt j n", j=2, n=T)
                    )
                else:
                    xc = xp.tile([128, KT, T], BF16)
                    nc.gpsimd.dma_start(
                        xc[:], xT8[b].rearrange("p (kt n) -> p kt n", n=T)
                    )
                tz_tiles = []
                for m in range(MT):
                    z_ps = pp.tile([128, T], F32, tag="ps")
                    if FP8_BIG:
                        for kt in range(KT2):
                            nc.tensor.matmul(
                                z_ps[:],
                                wm_sb[m][:, kt, :, :],
                                xc[:, kt, :, :],
                                start=(kt == 0),
                                stop=(kt == KT2 - 1),
                                perf_mode=mybir.MatmulPerfMode.DoubleRow,
                            )
                    else:
                        for kt in range(KT):
                            nc.tensor.matmul(
                                z_ps[:],
                                wm_sb[m][:, kt, :],
                                xc[:, kt, :],
                                start=(kt == 0),
                                stop=(kt == KT - 1),
                            )
                    tz = tzp.tile([128, T], BF16)
                    nc.scalar.activation(
                        tz[:],
                        z_ps[:],
                        TANH,
                        bias=bh_sb[:, m : m + 1],
                        scale=(1.0 / WSCALE) if FP8_BIG else 1.0,
                    )
                    tz_tiles.append(tz)
                s_ps = pp.tile([1, T], F32, tag="ps")
                for m in range(MT):
                    nc.tensor.matmul(
                        s_ps[:1, :],
                        u_sb[:, m : m + 1],
                        tz_tiles[m][:],
                        start=(m == 0),
                        stop=(m == MT - 1),
                    )
                # softmax for this batch on partition 0
                sc_b = scp.tile([1, T], F32, tag="sc")
                nc.scalar.copy(sc_b[:1, :], s_ps[:1, :])
                e_b = scp.tile([1, T], F32, tag="eb")
                nc.scalar.activation(e_b[:1, :], sc_b[:1, :], EXP)
                nc.vector.reduce_sum(
                    esum1[:1, b : b + 1], e_b[:1, :], axis=mybir.AxisListType.X
                )
                nc.vector.reciprocal(einv1[:1, b : b + 1], esum1[:1, b : b + 1])
                a_b = scp.tile([1, T], BF16, tag="ab")
                nc.vector.tensor_scalar(
                    a_b[:1, :],
                    e_b[:1, :],
                    einv1[:1, b : b + 1],
                    ASCALE,
                    mybir.AluOpType.mult,
                    mybir.AluOpType.mult,
                )
                # transpose alpha_b (stride-2 pairs) into masked fp8 columns
                for kt in range(TT2):
                    for j in range(2):
                        t_ps = tpp.tile([128, PB], BF16, tag="tp")
                        nc.tensor.transpose(
                            t_ps[:, :1],
                            a_b[:1, kt * 256 + j : (kt + 1) * 256 : 2],
                            id_sb[:1, :1],
                        )
                        nc.scalar.copy(
                            am_sb[:, kt, j, b, b : b + 1], t_ps[:, :1]
                        )
                # previous batch's r (its hidden had a full chunk to arrive)
                if prev is not None:
                    emit_r(*prev)
                hb_t = hbp.tile([128, TT2, 2, H], FP8)
                nc.sync.dma_start(
                    hb_t[:],
                    hQ8[b].rearrange("p (kt j h) -> p kt j h", j=2, h=H),
                )
                prev = (b, hb_t)
            emit_r(*prev)

            # ---- late-loaded constants (sync queue, drain during phase A) ----
            wpT_sb = cp.tile([128, KT, H], BF16)
            nc.sync.dma_start(wpT_sb[:], wpT[:].rearrange("(kt p) n -> p kt n", p=128))
            wxh_sb = cp.tile([128, KT, H], BF16)
            nc.sync.dma_start(wxh_sb[:], wxh[:].rearrange("(kt p) n -> p kt n", p=128))
            wxl_sb = cp.tile([128, KT, H], BF16)
            nc.sync.dma_start(wxl_sb[:], wxl[:].rearrange("(kt p) n -> p kt n", p=128))
            hlh_sb = cp.tile([128, KT, B], BF16)
            nc.sync.dma_start(hlh_sb[:], hlh[:].rearrange("(kt p) j -> p kt j", p=128))
            hll_sb = cp.tile([128, KT, B], BF16)
            nc.sync.dma_start(hll_sb[:], hll[:].rearrange("(kt p) j -> p kt j", p=128))
            selA_sb = cp.tile([PB, 4, 128], BF16)
            nc.sync.dma_start(selA_sb[:], selA[:])
            bpx_sb = cp.tile([1, 2 * H], BF16)
            nc.sync.dma_start(bpx_sb[:], bpx[:])
            ones_sb = cp.tile([1, B], BF16)
            nc.sync.dma_start(ones_sb[:], ones[:])

            # ---- phase F: x2 = (hlast @ W_x.T + b_p + b_x) in split bf16 ----
            x2_sb = sp.tile([128, H], F32)
            for hc in range(2):
                x_ps = pp.tile([B, 512], F32, tag="ps")
                n = 0
                terms = [(hlh_sb, wxh_sb), (hll_sb, wxh_sb), (hlh_sb, wxl_sb)]
                nmm = len(terms) * KT + 2
                for lh, rh in terms:
                    for kt in range(KT):
                        nc.tensor.matmul(
                            x_ps[:],
                            lh[:, kt, :],
                            rh[:, kt, hc * 512 : (hc + 1) * 512],
                            start=(n == 0),
                            stop=(n == nmm - 1),
                        )
                        n += 1
                for row in range(2):
                    nc.tensor.matmul(
                        x_ps[:],
                        ones_sb[:1, :],
                        bpx_sb[:1, row * H + hc * 512 : row * H + (hc + 1) * 512],
                        start=(n == 0),
                        stop=(n == nmm - 1),
                    )
                    n += 1
                nc.scalar.copy(x2_sb[:B, hc * 512 : (hc + 1) * 512], x_ps[:])
                nc.scalar.copy(x2_sb[B:, hc * 512 : (hc + 1) * 512], x_ps[:])

            # ---- r -> rT -> p ----
            rflat_bf = sp.tile([PB, H], BF16)
            for hc in range(2):
                nc.scalar.activation(
                    rflat_bf[:, hc * 512 : (hc + 1) * 512],
                    r_ps[hc][:],
                    mybir.ActivationFunctionType.Copy,
                    bias=0.0,
                    scale=1.0 / ASCALE,
                )
            rT_sb = sp.tile([128, KT, PB], BF16)
            for mt in range(KT):
                t_ps = tpp.tile([128, PB], BF16, tag="tp")
                nc.tensor.transpose(
                    t_ps[:], rflat_bf[:, mt * 128 : (mt + 1) * 128], id_sb[:]
                )
                nc.scalar.copy(rT_sb[:, mt, :], t_ps[:])
            p_sb = sp.tile([PB, H], BF16)
            for hc in range(2):
                p_ps = pp.tile([PB, 512], F32, tag="ps")
                for kt in range(KT):
                    nc.tensor.matmul(
                        p_ps[:],
                        rT_sb[:, kt, :],
                        wpT_sb[:, kt, hc * 512 : (hc + 1) * 512],
                        start=(kt == 0),
                        stop=(kt == KT - 1),
                    )
                nc.scalar.copy(p_sb[:, hc * 512 : (hc + 1) * 512], p_ps[:])

            # ---- phase G: out = tanh(A_sel @ p + x2) ----
            for q in range(4):
                for hc in range(2):
                    o_ps = pp.tile([128, 512], F32, tag="ps")
                    nc.tensor.matmul(
                        o_ps[:],
                        selA_sb[:, q, :],
                        p_sb[:, hc * 512 : (hc + 1) * 512],
                        start=True,
                        stop=True,
                    )
                    o_sb = op_.tile([128, 512], F32, tag="oadd")
                    nc.vector.tensor_add(
                        o_sb[:], o_ps[:], x2_sb[:, hc * 512 : (hc + 1) * 512]
                    )
                    o_sb2 = op_.tile([128, 512], F32, tag="otanh")
                    nc.scalar.activation(o_sb2[:], o_sb[:], TANH)
                    nc.sync.dma_start(
                        out[2 * q : 2 * q + 2, :, hc * 512 : (hc + 1) * 512].rearrange(
                            "i j h -> (i j) h"
                        ),
                        o_sb2[:],
                    )
    _split_excess_waits(nc)
    return nc


def _split_excess_waits(nc: bass.Bass, max_waits: int = 1) -> None:
    """Walrus's per-instruction sync-wait slots are limited; move excess
    on_wait entries onto wait-only NoOps inserted just before the
    instruction (same engine, so ordering is preserved)."""
    for fn in nc.m.functions:
        for blk in fn.blocks:
            new = []
            for inst in blk.instructions:
                si = inst.sync_info
                waits = list(si.on_wait) if si is not None and si.on_wait else []
                if len(waits) > max_waits:
                    extra, keep = waits[:-max_waits], waits[-max_waits:]
                    for ci in range(0, len(extra), max_waits):
                        nop = mybir.InstNoOp(
                            name=f"{inst.name}-wsplit{ci}", ins=[], outs=[]
                        )
                        nop.engine = inst.engine
                        nop.sync_info = mybir.SyncInfo(
                            on_wait=extra[ci : ci + max_waits], on_update=[]
                        )
                        new.append(nop)
                    inst.sync_info = mybir.SyncInfo(
                        on_wait=keep, on_update=list(si.on_update or [])
                    )
                new.append(inst)
            blk.instructions[:] = new


def _split_bf16(a: np.ndarray) -> tuple[np.ndarray, np.ndarray]:
    hi = a.astype(BF16_NP)
    lo = (a - hi.astype(np.float32)).astype(BF16_NP)
    return hi, lo


def _host_prep(inputs: dict) -> list[dict]:
    hidden = np.asarray(inputs["hidden"], np.float32)
    W_h = np.asarray(inputs["W_h"], np.float32)
    b_h = np.asarray(inputs["b_h"], np.float32)
    w_w = np.asarray(inputs["w_w"], np.float32)
    W_p = np.asarray(inputs["W_p"], np.float32)
    b_p = np.asarray(inputs["b_p"], np.float32)
    W_x = np.asarray(inputs["W_x"], np.float32)
    b_x = np.asarray(inputs["b_x"], np.float32)

    selA = np.zeros((PB, 4, 128), np.float32)
    for q in range(4):
        for m in range(128):
            selA[2 * q + m // 64, q, m] = 1.0

    wxT = np.ascontiguousarray(W_x.T)
    wx_hi, wx_lo = _split_bf16(wxT)
    hlT = np.ascontiguousarray(hidden[:, -1, :].T)
    hl_hi, hl_lo = _split_bf16(hlT)
    bpx_hi, bpx_lo = _split_bf16((b_p + b_x).reshape(1, H))

    shared = {}
    if FP8_BIG:
        shared["whQ8"] = np.ascontiguousarray(
            (W_h.T * WSCALE)
            .reshape(KT2, 128, 2, MT, 128)
            .transpose(3, 1, 0, 2, 4)
            .reshape(MT, 128, KT2 * 2 * 128)
        ).astype(FP8_NP)
    else:
        shared["whT8"] = np.ascontiguousarray(
            W_h.T.reshape(KT, 128, MT, 128).transpose(2, 1, 0, 3).reshape(
                MT, 128, KT * 128
            )
        ).astype(BF16_NP)
    shared.update({

        "bh": np.ascontiguousarray(b_h.reshape(MT, 128).T),
        "u": np.ascontiguousarray(w_w[0, :H].reshape(MT, 128).T).astype(BF16_NP),
        "wpT": np.ascontiguousarray(W_p.T).astype(BF16_NP),
        "wxT_hi": wx_hi,
        "wxT_lo": wx_lo,
        "hlastT_hi": hl_hi,
        "hlastT_lo": hl_lo,
        "selA": selA.astype(BF16_NP),
        "bpx": np.concatenate([bpx_hi, bpx_lo], axis=1),
        "ones": np.ones((1, B), BF16_NP),
        "ident": np.eye(PB, dtype=np.float32).astype(BF16_NP),
    })

    in_maps = []
    for c in range(NCORES):
        flat = hidden[c * PB : (c + 1) * PB].reshape(R, H)
        m = dict(shared)
        if FP8_BIG:
            m["xQ8"] = np.ascontiguousarray(
                flat.reshape(PB, T, KT2, 128, 2)
                .transpose(0, 3, 2, 4, 1)
                .reshape(PB, 128, KT2 * 2 * T)
            ).astype(FP8_NP)
        else:
            m["xT8"] = np.ascontiguousarray(
                flat.reshape(PB, T, KT, 128).transpose(0, 3, 2, 1).reshape(
                    PB, 128, KT * T
                )
            ).astype(BF16_NP)
        m["hQ8"] = np.ascontiguousarray(
            flat.reshape(PB, TT2, 128, 2, H)
            .transpose(0, 2, 1, 3, 4)
            .reshape(PB, 128, TT2 * 2 * H)
        ).astype(FP8_NP)
        in_maps.append(m)
    return in_maps


def _ensure_ntff_hook() -> None:
    """The agent image's antenv lacks axon_hooks; register a shim module
    wired to the libaxon NTFF profile hook so trace=True works."""
    try:
        from antenv.axon_hooks import get_axon_ntff_profile_hook  # noqa: F401
        return
    except ImportError:
        pass
    import types
    import antenv
    from trn_agent_boot.trn_boot import _ntff_profile_via_ctypes

    mod = types.ModuleType("antenv.axon_hooks")
    holder = {"hook": _ntff_profile_via_ctypes("/opt/axon/libaxon_pjrt.so")}
    mod.get_axon_ntff_profile_hook = lambda: holder["hook"]
    mod.set_axon_ntff_profile_hook = lambda h: holder.__setitem__("hook", h)
    sys.modules["antenv.axon_hooks"] = mod
    antenv.axon_hooks = mod


def run(inputs: dict, trace: bool = False, **kw):
    if trace:
        _ensure_ntff_hook()
    if "nc" not in _CACHE:
        _CACHE["nc"] = _build_nc()
    nc = _CACHE["nc"]
    in_maps = _host_prep(inputs)
    res = run_bass_kernel_spmd(nc, in_maps, list(range(NCORES)), trace=trace, **kw)
    out = np.empty((B, B, H), np.float32)
    for c in range(NCORES):
        out[c * PB : (c + 1) * PB] = np.asarray(res.results[c]["out"], np.float32)
    return out, res


def kernel(**inputs) -> np.ndarray:
    out, _ = run(inputs)
    return out
